# revision 15
# baseline (speedup 1.0000x reference)
"""Trainium2 Bass kernel v3: single transformer layer (attn + gated MLP, LoRA on
all projections), B=4 S=1024 D=2048 H=16 HD=128 FF=8192, fp32 in/out.

Sharding (8 cores, no collectives): core c -> batch b=c//2, q-row chunks
{0,3} (c%2==0) or {1,2} (c%2==1) of 256 rows each (causally balanced).

vs v2: all seven projection GEMMs run as fp8e4 DoubleRow matmuls (2 k-tiles
per instruction at 0.5 cycles/row) with a 3-pass hi/lo error-compensation
scheme  W@x ~= Wh@xh + Wl@xh + Wh@xl  (each operand split into an fp8 "hi"
part plus an fp8 residual "lo" part at the same power-of-2 scale), which is
0.75x the PE cycles of the fp16 baseline at ~fp16-level accuracy.
Attention (scores/AV), RoPE rotation and all transposes stay fp16; softmax,
norms and residuals stay fp32. Dequantization uses fixed power-of-2 scales
folded into the RoPE tables, the softmax Exp, the Silu and epilogue copies.
"""
import numpy as np
import ml_dtypes
from contextlib import ExitStack

import concourse.bass as bass
import concourse.tile as tile
import concourse.mybir as mybir
from concourse import bacc
import concourse.bass_isa as bass_isa
from concourse.bass_utils import run_bass_kernel_spmd

F32 = mybir.dt.float32
F16 = mybir.dt.float16
F8 = mybir.dt.float8e4
AL = mybir.AluOpType
AF = mybir.ActivationFunctionType
PM = mybir.MatmulPerfMode

B, S, D = 4, 1024, 2048
H, HD = 16, 128
FF = 8192
EPS = 1e-5
CH = 256              # q-chunk rows
DB = D // 128         # 16
SB = S // 128         # 8
FFB = FF // 128       # 64
CHUNKS = [[0, 3], [1, 2]]  # global q-chunk ids per half

# quantization scales (all powers of two; dequants are compile-time consts)
SW = 256.0            # weight scale for q,k,v,o,gate,up (rms ~0.022 -> ~5.7)
SWD = 512.0           # weight scale for down (rms ~0.011 -> ~5.7)
SX = 8.0              # h1/h2 activation scale (rms 1)
SXO = 16.0            # oT activation scale (o rms ~0.14)
SXH = 8.0             # had activation scale (rms ~0.6, max ~16 -> 128)
ISQ = float(1.0 / np.sqrt(HD))

_CACHE = {}


def _dr(nc, psum, w8, x8h, x8l, kts, msl, start=True, stop=True, wsl=None):
    """3-pass hi/lo DoubleRow GEMM accumulation into psum.

    w8:   [128, 2(hi/lo), KT, m] fp8 weight tile
    x8h/x8l: moving fp8 tensors, sliced as x[:, 2t:2t+2, msl]
    kts:  list of k-tile indices to process (must be even count, paired)

    NOTE: start=True zeroes the WHOLE psum bank (lazily, applied at each
    region's first write) — callers accumulating several sub-regions of one
    bank must pass start=True only for the very first sub-region.
    """
    if wsl is None:
        wsl = slice(None)
    passes = [(0, x8h), (1, x8h), (0, x8l)]
    np_ = len(passes)
    for pi, (wi, xm) in enumerate(passes):
        for ti in range(0, len(kts), 2):
            t0 = kts[ti]
            first = (pi == 0 and ti == 0)
            last = (pi == np_ - 1 and ti == len(kts) - 2)
            nc.tensor.matmul(psum, w8[:, wi, t0:t0 + 2, wsl],
                             xm[:, t0:t0 + 2, msl],
                             start=(start and first), stop=(stop and last),
                             perf_mode=PM.DoubleRow,
                             skip_group_check=not (start and first))


def _finish_oproj(nc, avps, identH, x1, x_re, n2stats, db, xp, xoT):
    """PSUM o-proj tile -> dequant fp16 -> transpose -> +x -> x1; norm2 stats."""
    dsl = slice(db * 128, (db + 1) * 128)
    nc.scalar.mul(xoT[:], xp[:], 1.0 / (SW * SXO))
    tpf = avps.tile([128, 2, 256], F32, tag="op")
    tp = tpf[:, 0:1, :].bitcast(F16).rearrange("p a (j m) -> p (a j) m", m=128)
    for j in range(4):
        nc.tensor.transpose(tp[:, j, :], xoT[:, j * 128:(j + 1) * 128], identH[:])
    for tg in range(4):
        nc.vector.tensor_tensor(out=x1[:, tg, dsl], in0=tp[:, tg, :],
                                in1=x_re[tg][:, dsl], op=AL.add)
        nc.vector.bn_stats(out=n2stats[:, tg, db, :], in_=x1[:, tg, dsl])


def _finish_down(nc, tps2, identH, x1, outp, ap, db, dp, x2c, split=False):
    """PSUM down tile -> dequant fp16 -> transpose -> +x1 -> out DMA."""
    dsl = slice(db * 128, (db + 1) * 128)
    odb = outp.tile([128, 4, 128], F32, tag="odb")
    outv = ap["out"][db].rearrange("(g p) m -> p g m", p=128)
    for half in ([0] if not split else [0, 1]):
        hw = 4 if not split else 2
        gsl = slice(half * hw, half * hw + hw)
        csl = slice(half * hw * 128, (half * hw + hw) * 128)
        nc.scalar.mul(x2c[:, csl], dp[:, csl], 1.0 / (SWD * SXH))
        tp = tps2.tile([128, hw, 128], F16, tag="tp5")
        for j in range(hw):
            g = half * hw + j
            nc.tensor.transpose(tp[:, j, :], x2c[:, g * 128:(g + 1) * 128], identH[:])
        nc.vector.tensor_tensor(out=odb[:, gsl, :], in0=tp[:],
                                in1=x1[:, gsl, dsl], op=AL.add)
        nc.sync.dma_start(out=outv[:, gsl, :], in_=odb[:, gsl, :])


def _emit(nc, tc, ap, half):
    chunks = CHUNKS[half]
    own_rb = [rb for qc in chunks for rb in (2 * qc, 2 * qc + 1)]

    ctx = ExitStack()
    with ctx:
        persist = ctx.enter_context(tc.tile_pool(name="persist", bufs=1))

        eps_t = persist.tile([128, 1], F32)
        nc.vector.memset(eps_t, EPS / (SX * SX))
        identH = persist.tile([128, 128], F16)
        rotT = persist.tile([128, 128], F16)
        cosT = persist.tile([HD, S], F16)     # pre-scaled by 1/(SW*SX)
        sinT = persist.tile([HD, S], F16)
        maskT = persist.tile([128, 2, 2, CH], F32)

        gbig = ctx.enter_context(tc.tile_pool(name="gbig", bufs=1))
        x1 = gbig.tile([128, 4, D], F16)         # 2MB fp16 (own 512 rows)

        n2stats = gbig.tile([128, 4, DB, 6], F32)
        mw = ctx.enter_context(tc.tile_pool(name="mw", bufs=4))

        # ---- head loop pools (weight pool opened early for prefetch) ----
        hlA = ExitStack()
        wpool = hlA.enter_context(tc.tile_pool(name="wpool", bufs=3))
        small = hlA.enter_context(tc.tile_pool(name="small", bufs=2))
        kqps = hlA.enter_context(tc.tile_pool(name="kqps", bufs=4, space="PSUM"))
        avps = hlA.enter_context(tc.tile_pool(name="avps", bufs=1, space="PSUM"))
        opool = hlA.enter_context(tc.tile_pool(name="opool", bufs=1))
        oTh = opool.tile([128, H, 512], F8)      # 1MB (16*o, hi)
        oTl = opool.tile([128, H, 512], F8)      # 1MB (16*o, lo)
        wop = hlA.enter_context(tc.tile_pool(name="wop", bufs=3))
        xop = hlA.enter_context(tc.tile_pool(name="xop", bufs=4))
        wo_tiles = {}

        def wo_dma(db):
            wo_t = wop.tile([128, 2, H, 128], F8, tag="wo")
            nc.sync.dma_start(
                out=wo_t,
                in_=ap["wo"][db].rearrange("p (two hh m) -> p two hh m",
                                           two=2, m=128))
            wo_tiles[db] = wo_t

        gu_tiles = {}

        def gu_dma(fb):
            wg_t = mw.tile([128, 2, DB, 128], F8, tag="wg")
            wu_t = mw.tile([128, 2, DB, 128], F8, tag="wu")
            nc.sync.dma_start(out=wg_t, in_=ap["wg"][fb].rearrange(
                "p (two db m) -> p two db m", two=2, m=128))
            nc.sync.dma_start(out=wu_t, in_=ap["wu"][fb].rearrange(
                "p (two db m) -> p two db m", two=2, m=128))
            gu_tiles[fb] = (wg_t, wu_t)

        hlB = ExitStack()
        vps = hlB.enter_context(tc.tile_pool(name="vps", bufs=1, space="PSUM"))
        abig = hlB.enter_context(tc.tile_pool(name="abig", bufs=1))
        h1Th = abig.tile([128, DB, S], F8)       # 2MB (8*h1, hi)
        h1Tl = abig.tile([128, DB, S], F8)       # 2MB (8*h1, lo)

        wtiles = {}

        def wdma(h):
            wk_t = wpool.tile([128, 2, DB, 128], F8, tag="wk")
            wv_t = wpool.tile([128, 2, DB, 128], F8, tag="wv")
            wq_t = wpool.tile([128, 2, DB, 128], F8, tag="wq")
            for t, nm in ((wk_t, "wk"), (wv_t, "wv"), (wq_t, "wq")):
                nc.sync.dma_start(out=t, in_=ap[nm][h].rearrange(
                    "p (two db m) -> p two db m", two=2, m=128))
            wtiles[h] = (wk_t, wv_t, wq_t)

        def _kmm(g, wk_t):
            ssl = slice(g * 512, (g + 1) * 512)
            kp = kqps.tile([128, 512], F32, tag="kp")
            _dr(nc, kp[:], wk_t, h1Th, h1Tl, list(range(DB)), ssl,
                start=True, stop=False)
            tsin = small.tile([128, 512], F16, tag="tsin")
            nc.vector.tensor_tensor(out=tsin[:], in0=kp[:], in1=sinT[:, ssl], op=AL.mult)
            nc.vector.tensor_tensor(out=kp[:], in0=kp[:], in1=cosT[:, ssl], op=AL.mult)
            return kp, tsin

        def _vmm(vb, wv_t):
            # tokens stationary, weights moving -> natural [token, hd] layout
            vp = vps.tile([128, 4, 128], F32, tag="vp")
            for j in range(4):
                blk = vb * 4 + j
                bsl = slice(blk * 128, (blk + 1) * 128)
                for pi, (wi, xm) in enumerate([(0, h1Th), (1, h1Th), (0, h1Tl)]):
                    for t0 in range(0, DB, 2):
                        nc.tensor.matmul(
                            vp[:, j, :], xm[:, t0:t0 + 2, bsl],
                            wv_t[:, wi, t0:t0 + 2, :],
                            start=(pi == 0 and t0 == 0),
                            stop=(pi == 2 and t0 == DB - 2),
                            perf_mode=PM.DoubleRow, skip_group_check=True)
            return vp

        # ---- phase A: rmsnorm1 -> h1 (fp16, 8x scale) -> transpose ->
        #      split into h1Th/h1Tl fp8 ----
        pres = {0: {}, 1: {}, 2: {}}
        nc.sync.dma_start(out=identH, in_=ap["identH"])
        with tc.tile_pool(name="stA", bufs=2) as stA, \
             tc.tile_pool(name="stAx", bufs=1) as stAx, \
             tc.tile_pool(name="stAps", bufs=2, space="PSUM") as stAps:
            xown = {}
            for rb in [0, 1, 2, 3, "pre", 4, 5, "pre2", 6, 7]:
                if rb == "pre":
                    # head-0/1/2 partial work on tokens 0-511 fills the PE
                    # while the rest of phase A streams through DVE/Act/Pool
                    for hh in (0, 1):
                        pres[hh]["w"] = wtiles.pop(hh)
                        pres[hh]["kp0"], pres[hh]["tsin0"] = \
                            _kmm(0, pres[hh]["w"][0])
                    pres[0]["vp0"] = _vmm(0, pres[0]["w"][1])
                    continue
                if rb == "pre2":
                    pres[2]["w"] = wtiles.pop(2)
                    pres[2]["kp0"], pres[2]["tsin0"] = _kmm(0, pres[2]["w"][0])
                    continue
                if rb in own_rb:
                    x_t = xop.tile([128, D], F16, tag="xot")
                    xown[own_rb.index(rb)] = x_t
                else:
                    x_t = stA.tile([128, D], F16, tag="x_t")
                nc.sync.dma_start(out=x_t, in_=ap["xbh"][rb * 128:(rb + 1) * 128, :])
                xsq = stAx.tile([128, D], F16, tag="xsq")
                acc = stA.tile([128, 1], F32, tag="acc")
                if rb == 0:
                    wdma(0)
                elif rb == 1:
                    nc.sync.dma_start(out=rotT, in_=ap["rotT"])
                    nc.sync.dma_start(out=cosT, in_=ap["cosT"])
                    nc.sync.dma_start(out=sinT, in_=ap["sinT"])
                elif rb == 2:
                    wdma(1)
                elif rb == 3:
                    nc.sync.dma_start(out=maskT,
                                      in_=ap["maskT"].rearrange("c k p q -> p c k q"))
                    wdma(2)
                nc.scalar.activation(out=xsq[:], in_=x_t[:], func=AF.Square,
                                     accum_out=acc[:])
                # sd = sqrt(mean(x^2)+eps)/SX ; rstd = SX/sd scale folded in
                sd = stA.tile([128, 1], F32, tag="sdA")
                nc.scalar.activation(out=sd[:], in_=acc[:], func=AF.Sqrt,
                                     scale=float(1.0 / (D * SX * SX)), bias=eps_t[:])
                rstd = stA.tile([128, 1], F32, tag="rstdA")
                nc.vector.reciprocal(out=rstd[:], in_=sd[:])
                h1 = stA.tile([128, D], F16, tag="h1")
                for hk in range(2):
                    hsl = slice(hk * 1024, (hk + 1) * 1024)
                    if hk == 0:
                        nc.scalar.mul(h1[:, hsl], x_t[:, hsl], rstd[:])
                    else:
                        nc.vector.tensor_scalar_mul(h1[:, hsl], x_t[:, hsl], rstd[:])
                    for pk in range(2 * hk, 2 * hk + 2):
                        tp = stAps.tile([128, 4, 128], F16, tag="tpA")
                        for j in range(4):
                            nc.tensor.transpose(
                                tp[:, j, :],
                                h1[:, (4 * pk + j) * 128:(4 * pk + j + 1) * 128],
                                identH[:])
                        rsl = slice(rb * 128, (rb + 1) * 128)
                        dsth = h1Th[:, 4 * pk:4 * pk + 4, rsl]
                        nc.scalar.copy(dsth, tp[:])
                        nc.vector.tensor_tensor(
                            out=h1Tl[:, 4 * pk:4 * pk + 4, rsl],
                            in0=tp[:], in1=dsth, op=AL.subtract)

        kpool = hlB.enter_context(tc.tile_pool(name="kpool", bufs=2))
        ppool = hlB.enter_context(tc.tile_pool(name="ppool", bufs=2))
        sps = hlB.enter_context(tc.tile_pool(name="sps", bufs=2, space="PSUM"))

        hstate = {}

        def _own_msl(qc):
            return slice(qc * CH, (qc + 1) * CH)

        def proj(h, pre=None, mid=None):
            if pre is None:
                wk_t, wv_t, wq_t = wtiles.pop(h)
            else:
                wk_t, wv_t, wq_t = pre["w"]

            kT_h = kpool.tile([128, S], F16, tag="kT")
            v_nat = kpool.tile([128, SB, 128], F16, tag="v_nat")
            qT_h = kpool.tile([128, 512], F16, tag="qT")

            def krot(g, kp, tsin):
                ssl = slice(g * 512, (g + 1) * 512)
                nc.tensor.matmul(kp[:], rotT[:], tsin[:],
                                 start=False, stop=True, skip_group_check=True)
                nc.scalar.copy(kT_h[:, ssl], kp[:])

            if pre is None:
                kp0, tsin0 = _kmm(0, wk_t)
                vp0 = _vmm(0, wv_t)
            else:
                kp0, tsin0 = pre["kp0"], pre["tsin0"]
                vp0 = pre.get("vp0")
                if vp0 is None:
                    vp0 = _vmm(0, wv_t)
            kp1, tsin1 = _kmm(1, wk_t)
            if mid is not None:
                mid()
            krot(0, kp0, tsin0)
            nc.scalar.mul(v_nat[:, 0:4, :], vp0[:], float(SXO / (SW * SX)))

            # q matmuls (own chunks), DoubleRow per chunk
            qp = kqps.tile([128, 512], F32, tag="kp")
            qpv = qp[:].rearrange("p (a c) -> p a c", c=CH)
            for ci, qc in enumerate(chunks):
                # one spanning accumulation group: only the very first matmul
                # carries start=True (bank-wide lazy zero covers chunk 1)
                _dr(nc, qpv[:, ci, :], wq_t, h1Th, h1Tl, list(range(DB)),
                    _own_msl(qc), start=(ci == 0), stop=False)
            krot(1, kp1, tsin1)
            tsin = small.tile([128, 512], F16, tag="tsin")
            tsv = tsin[:].rearrange("p (a c) -> p a c", c=CH)
            for ci, qc in enumerate(chunks):
                osl = _own_msl(qc)
                nc.vector.tensor_tensor(out=tsv[:, ci, :], in0=qpv[:, ci, :],
                                        in1=sinT[:, osl], op=AL.mult)
                nc.vector.tensor_tensor(out=qpv[:, ci, :], in0=qpv[:, ci, :],
                                        in1=cosT[:, osl], op=AL.mult)

            vp1 = _vmm(1, wv_t)
            nc.tensor.matmul(qp[:], rotT[:], tsin[:],
                             start=False, stop=True, skip_group_check=True)
            nc.scalar.copy(qT_h[:], qp[:])
            nc.vector.tensor_scalar_mul(v_nat[:, 4:8, :], vp1[:],
                                        float(SXO / (SW * SX)))
            if h + 3 < H:
                wdma(h + 3)
            if h >= 13:
                wo_dma(h - 13)
            hstate[h] = (kT_h, v_nat, qT_h)

        def attn_scores(h):
            kT_h, v_nat, qT_h = hstate.pop(h)
            pcs = []
            for ci, qc in enumerate(chunks):
                nkb = 2 * qc + 2
                osl = slice(ci * CH, (ci + 1) * CH)
                p_sb = ppool.tile([128, SB, CH], F16, tag="p_sb")
                for kb in range(nkb):
                    sp = sps.tile([128, CH], F32, tag="sp")
                    nc.tensor.matmul(sp[:], kT_h[:, kb * 128:(kb + 1) * 128],
                                     qT_h[:, osl], start=True, stop=True,
                                     skip_group_check=True)
                    j = kb - 2 * qc
                    if j >= 0:
                        nc.vector.scalar_tensor_tensor(
                            out=sp[:], in0=sp[:], scalar=ISQ,
                            in1=maskT[:, ci, j, :], op0=AL.mult, op1=AL.add)
                        nc.scalar.activation(out=p_sb[:, kb, :], in_=sp[:],
                                             func=AF.Exp)
                    else:
                        nc.scalar.activation(out=p_sb[:, kb, :], in_=sp[:],
                                             func=AF.Exp, scale=ISQ)
                den = small.tile([128, CH], F32, tag="den")
                nc.vector.tensor_tensor(out=den[:], in0=p_sb[:, 0, :],
                                        in1=p_sb[:, 1, :], op=AL.add)
                for kb in range(2, nkb):
                    nc.vector.tensor_tensor(out=den[:], in0=den[:],
                                            in1=p_sb[:, kb, :], op=AL.add)
                dall = small.tile([128, CH], F32, tag="dall")
                nc.gpsimd.partition_all_reduce(dall[:], den[:], 128,
                                               bass_isa.ReduceOp.add)
                dbc = small.tile([128, CH], F32, tag="dbc")
                nc.vector.reciprocal(out=dbc[:], in_=dall[:])
                pcs.append((nkb, osl, p_sb, dbc))
            return v_nat, pcs

        def attn_av(h, st):
            v_nat, pcs = st
            op_ = avps.tile([128, 2, CH], F32, tag="op")
            for ci in range(2):
                nkb, osl, p_sb, dbc = pcs[ci]
                for kb in range(nkb):
                    nc.tensor.matmul(op_[:, ci, :], v_nat[:, kb, :], p_sb[:, kb, :],
                                     start=(kb == 0), stop=(kb == nkb - 1),
                                     skip_group_check=True)
                t16 = small.tile([128, CH], F16, tag="t16")
                nc.vector.tensor_tensor(out=t16[:], in0=op_[:, ci, :],
                                        in1=dbc[:], op=AL.mult)
                nc.gpsimd.tensor_copy(oTh[:, h, osl], t16[:])
                nc.vector.tensor_tensor(out=oTl[:, h, osl], in0=t16[:],
                                        in1=oTh[:, h, osl], op=AL.subtract)

        ast = {}

        for h in range(H):
            def mid(hh=h):
                if hh > 0:
                    ast[hh - 1] = attn_scores(hh - 1)
            proj(h, pres.get(h) or None, mid=mid)
            if h > 0:
                attn_av(h - 1, ast.pop(h - 1))
        ast[H - 1] = attn_scores(H - 1)
        attn_av(H - 1, ast.pop(H - 1))
        hlB.close()

        # ---- o-proj + residual -> x1 (SBUF) + incremental norm2 stats ----
        with tc.tile_pool(name="st3", bufs=2) as st3:
            pending = None
            for db in range(DB):
                if db + 3 < DB:
                    wo_dma(db + 3)
                if db in (2, 6, 10, 13):
                    gu_dma({2: 0, 6: 1, 10: 2, 13: 3}[db])
                wo_t = wo_tiles.pop(db)
                xp = kqps.tile([128, 512], F32, tag="kp")
                _dr(nc, xp[:], wo_t, oTh, oTl, list(range(H)), slice(None))
                if pending is not None:
                    _finish_oproj(nc, avps, identH, x1, xown, n2stats, *pending)
                xoT = st3.tile([128, 512], F16, tag="xoT")
                pending = (db, xp, xoT)
            _finish_oproj(nc, avps, identH, x1, xown, n2stats, *pending)
        hlA.close()

        mlp = ctx.enter_context(tc.tile_pool(name="mlp", bufs=1))
        h2Th = mlp.tile([128, DB, 512], F8)      # 1MB (8*h2, hi)
        h2Tl = mlp.tile([128, DB, 512], F8)      # 1MB (8*h2, lo)
        hadh = mlp.tile([128, FFB, 512], F8)     # 4MB (8*had, hi)
        hadl = mlp.tile([128, FFB, 512], F8)     # 4MB (8*had, lo)

        # ---- norm2 finalize -> h2Th/h2Tl (fp8, 8x scale) ----
        with tc.tile_pool(name="st4", bufs=4) as st4, \
             tc.tile_pool(name="nps", bufs=4, space="PSUM") as nps:
            diags = []
            for tg in range(4):
                mv = st4.tile([128, 2], F32, tag="n2mv")
                nc.vector.bn_aggr(out=mv[:], in_=n2stats[:, tg, :, :])
                msq = st4.tile([128, 1], F32, tag="n2msq")
                nc.vector.scalar_tensor_tensor(
                    out=msq[:], in0=mv[:, 0:1], scalar=mv[:, 0:1], in1=mv[:, 1:2],
                    op0=AL.mult, op1=AL.add)
                sd = st4.tile([128, 1], F32, tag="n2sd")
                nc.scalar.activation(out=sd[:], in_=msq[:], func=AF.Sqrt,
                                     scale=float(1.0 / (SX * SX)), bias=eps_t[:])
                rstd = st4.tile([128, 1], F32, tag="n2rstd")
                nc.vector.reciprocal(out=rstd[:], in_=sd[:])
                # diag(SX*rstd) fp16; x1_slice^T @ diag transposes + norm-scales
                diag = st4.tile([128, 128], F16, tag="n2diag")
                nc.vector.tensor_scalar_mul(diag[:], identH[:], rstd[:])
                diags.append(diag)
            # pk-major so h2T k-tiles complete in db order
            for pk in range(4):
                for tg in range(4):
                    tp = nps.tile([128, 4, 128], F32, tag="tpN")
                    for j in range(4):
                        nc.tensor.matmul(
                            tp[:, j, :],
                            x1[:, tg, (4 * pk + j) * 128:(4 * pk + j + 1) * 128],
                            diags[tg][:], start=True, stop=True,
                            skip_group_check=True)
                    tsl = slice(tg * 128, (tg + 1) * 128)
                    dsth = h2Th[:, 4 * pk:4 * pk + 4, tsl]
                    nc.scalar.copy(dsth, tp[:])
                    nc.vector.tensor_tensor(out=h2Tl[:, 4 * pk:4 * pk + 4, tsl],
                                            in0=tp[:], in1=dsth, op=AL.subtract)

        # ---- MLP gate/up -> hadh/hadl ----
        wdp_cm = tc.tile_pool(name="wd", bufs=2)
        wdp = wdp_cm.__enter__()
        wd_tiles = {}

        def wd_dma(db):
            wd_t = wdp.tile([128, 2, FFB, 128], F8, tag="wd")
            nc.sync.dma_start(out=wd_t, in_=ap["wd"][db].rearrange(
                "p (two fb m) -> p two fb m", two=2, m=128))
            wd_tiles[db] = wd_t

        with tc.tile_pool(name="mls", bufs=3) as mls, \
             tc.tile_pool(name="gps", bufs=2, space="PSUM") as gps, \
             tc.tile_pool(name="ups", bufs=2, space="PSUM") as ups:
            for fb in range(FFB):
                if fb + 4 < FFB:
                    gu_dma(fb + 4)
                elif fb == FFB - 4:
                    wd_dma(0)
                elif fb == FFB - 2:
                    wd_dma(1)
                wg_t, wu_t = gu_tiles.pop(fb)
                gp = gps.tile([128, 512], F32, tag="gp")
                up = ups.tile([128, 512], F32, tag="up")
                _dr(nc, gp[:], wg_t, h2Th, h2Tl, list(range(DB)), slice(None))
                _dr(nc, up[:], wu_t, h2Th, h2Tl, list(range(DB)), slice(None))
                sg = mls.tile([128, 512], F16, tag="sg")
                nc.scalar.activation(out=sg[:], in_=gp[:], func=AF.Silu,
                                     scale=float(1.0 / (SW * SX)))
                t16 = mls.tile([128, 512], F16, tag="t16m")
                nc.vector.scalar_tensor_tensor(
                    out=t16[:], in0=up[:], scalar=float(SXH / (SW * SX)),
                    in1=sg[:], op0=AL.mult, op1=AL.mult)
                nc.gpsimd.tensor_copy(hadh[:, fb, :], t16[:])
                nc.vector.tensor_tensor(out=hadl[:, fb, :], in0=t16[:],
                                        in1=hadh[:, fb, :], op=AL.subtract)
        # ---- MLP down + final residual -> out ----
        with tc.tile_pool(name="st5", bufs=2) as st5, \
             tc.tile_pool(name="outp", bufs=2) as outp, \
             tc.tile_pool(name="dps", bufs=2, space="PSUM") as dps, \
             tc.tile_pool(name="tps2", bufs=2, space="PSUM") as tps2:
            pending = None
            for db in range(DB):
                if db + 2 < DB:
                    wd_dma(db + 2)
                wd_t = wd_tiles.pop(db)
                dp = dps.tile([128, 512], F32, tag="dp")
                _dr(nc, dp[:], wd_t, hadh, hadl, list(range(FFB)), slice(None))
                if pending is not None:
                    _finish_down(nc, tps2, identH, x1, outp, ap, *pending)
                x2c = st5.tile([128, 512], F16, tag="x2c")
                pending = (db, dp, x2c)
            _finish_down(nc, tps2, identH, x1, outp, ap, *pending, split=True)
        wdp_cm.__exit__(None, None, None)


def _build(half):
    nc = bacc.Bacc("TRN2", target_bir_lowering=False, debug=False, num_devices=8)
    ap = {}

    def din(name, shape, dt=F8):
        ap[name] = nc.dram_tensor(name, shape, dt, kind="ExternalInput").ap()

    din("xbh", [S, D], F16)
    din("cosT", [HD, S], F16)
    din("sinT", [HD, S], F16)
    din("maskT", [2, 2, 128, CH], F32)
    din("identH", [128, 128], F16)
    din("rotT", [128, 128], F16)
    din("wq", [H, 128, 2 * DB * 128]); din("wk", [H, 128, 2 * DB * 128])
    din("wv", [H, 128, 2 * DB * 128])
    din("wo", [DB, 128, 2 * H * 128])
    din("wg", [FFB, 128, 2 * DB * 128]); din("wu", [FFB, 128, 2 * DB * 128])
    din("wd", [DB, 128, 2 * FFB * 128])
    ap["out"] = nc.dram_tensor("out", [DB, 512, 128], F32, kind="ExternalOutput").ap()

    with tile.TileContext(nc) as tc:
        _emit(nc, tc, ap, half)
    nc.compile()
    return nc


def _q8(a):
    return np.asarray(a, np.float32).astype(ml_dtypes.float8_e4m3)


def _hilo(w, s):
    """[.., K-tiles, m] float32 -> stacked hi/lo fp8 at scale s (axis -3)."""
    wh = _q8(w * s)
    wl = _q8(w * s - wh.astype(np.float32))
    return np.stack([wh, wl], axis=-3)


def _prep(inputs):
    inp = {k: np.asarray(v) for k, v in inputs.items()}
    w1 = inp["norm_weight_1"].astype(np.float32)
    w2 = inp["norm_weight_2"].astype(np.float32)

    def fold(n):
        return (inp[f"w_{n}"].astype(np.float32)
                + inp[f"w_{n}_lora_a"].astype(np.float32)
                @ inp[f"w_{n}_lora_b"].astype(np.float32))

    ident = np.eye(128, dtype=np.float16)
    Rm = np.zeros((128, 128), np.float32)
    for i in range(64):
        Rm[i, i + 64] = -1.0
        Rm[i + 64, i] = 1.0

    def _colmajor8(w, nblk, s):
        # [K, N] -> [nblk, 128, 2*(K/128)*(N/nblk)] fp8 hi/lo
        K, N = w.shape
        r = (w.reshape(K // 128, 128, nblk, N // nblk)
             .transpose(2, 1, 0, 3))          # [nblk, 128, KT, n]
        hl = _hilo(r, s)                      # [nblk, 128, 2, KT, n]
        return np.ascontiguousarray(
            hl.reshape(nblk, 128, 2 * (K // 128) * (N // nblk)))

    wo_f = fold("o")
    wo_r = wo_f.reshape(H, 128, DB, 128).transpose(2, 1, 0, 3)  # [DB,128,H,128]
    wo_hl = np.ascontiguousarray(_hilo(wo_r, SW).reshape(DB, 128, 2 * H * 128))
    wd_f = fold("down")
    wd_r = wd_f.reshape(FFB, 128, DB, 128).transpose(2, 1, 0, 3)
    wd_hl = np.ascontiguousarray(_hilo(wd_r, SWD).reshape(DB, 128, 2 * FFB * 128))

    shared = dict(
        wq=_colmajor8(w1[:, None] * fold("q"), H, SW),
        wk=_colmajor8(w1[:, None] * fold("k"), H, SW),
        wv=_colmajor8(w1[:, None] * fold("v"), H, SW),
        wo=wo_hl,
        wg=_colmajor8(w2[:, None] * fold("gate"), FFB, SW),
        wu=_colmajor8(w2[:, None] * fold("up"), FFB, SW),
        wd=wd_hl,
        identH=ident, rotT=np.ascontiguousarray(Rm.T.astype(np.float16)))

    pos = inp["position_ids"].astype(np.int64)
    cs = float(1.0 / (SW * SX))
    cos_p = inp["cos"].astype(np.float32)[pos] * cs
    sin_p = inp["sin"].astype(np.float32)[pos] * cs
    mask = inp["attention_mask"].astype(np.float32)[0, 0]
    x = inp["x"].astype(np.float32)

    in_maps = []
    for c in range(8):
        b = c // 2
        half = c % 2
        mT = np.zeros((2, 2, 128, CH), np.float32)
        for ci, qc in enumerate(CHUNKS[half]):
            for j in range(2):
                kb = 2 * qc + j
                mT[ci, j] = mask[qc * CH:(qc + 1) * CH, kb * 128:(kb + 1) * 128].T
        m = dict(shared)
        m.update(xbh=np.ascontiguousarray(x[b].astype(np.float16)),
                 cosT=np.ascontiguousarray(cos_p[b].T.astype(np.float16)),
                 sinT=np.ascontiguousarray(sin_p[b].T.astype(np.float16)),
                 maskT=mT)
        in_maps.append(m)
    return in_maps


def kernel(**inputs):
    in_maps = _prep(inputs)
    if "nc" not in _CACHE:
        _CACHE["nc"] = (_build(0), _build(1))
    nc0, nc1 = _CACHE["nc"]

    res0 = run_bass_kernel_spmd(nc0, [in_maps[c] for c in (0, 2, 4, 6)],
                                core_ids=[0, 2, 4, 6])
    res1 = run_bass_kernel_spmd(nc1, [in_maps[c] for c in (1, 3, 5, 7)],
                                core_ids=[1, 3, 5, 7])

    out = np.zeros((B, S, D), np.float32)
    for res, halfi, cores in ((res0, 0, (0, 2, 4, 6)), (res1, 1, (1, 3, 5, 7))):
        for gi, c in enumerate(cores):
            b = c // 2
            r = res.results[gi]["out"]   # [DB, 512, 128]
            for ci, qc in enumerate(CHUNKS[halfi]):
                for db in range(DB):
                    out[b, qc * CH:(qc + 1) * CH, db * 128:(db + 1) * 128] = \
                        r[db, ci * CH:(ci + 1) * CH, :]
    return out


# revision 19
# speedup vs baseline: 1.0456x; 1.0456x over previous
"""Trainium2 Bass kernel v3: single transformer layer (attn + gated MLP, LoRA on
all projections), B=4 S=1024 D=2048 H=16 HD=128 FF=8192, fp32 in/out.

Sharding (8 cores, no collectives): core c -> batch b=c//2, q-row chunks
{0,3} (c%2==0) or {1,2} (c%2==1) of 256 rows each (causally balanced).

vs v2: all seven projection GEMMs run as fp8e4 DoubleRow matmuls (2 k-tiles
per instruction at 0.5 cycles/row) with a 3-pass hi/lo error-compensation
scheme  W@x ~= Wh@xh + Wl@xh + Wh@xl  (each operand split into an fp8 "hi"
part plus an fp8 residual "lo" part at the same power-of-2 scale), which is
0.75x the PE cycles of the fp16 baseline at ~fp16-level accuracy.
Attention (scores/AV), RoPE rotation and all transposes stay fp16; softmax,
norms and residuals stay fp32. Dequantization uses fixed power-of-2 scales
folded into the RoPE tables, the softmax Exp, the Silu and epilogue copies.
"""
import numpy as np
import ml_dtypes
from contextlib import ExitStack

import concourse.bass as bass
import concourse.tile as tile
import concourse.mybir as mybir
from concourse import bacc
import concourse.bass_isa as bass_isa
from concourse.bass_utils import run_bass_kernel_spmd

F32 = mybir.dt.float32
F16 = mybir.dt.float16
F8 = mybir.dt.float8e4
AL = mybir.AluOpType
AF = mybir.ActivationFunctionType
PM = mybir.MatmulPerfMode

B, S, D = 4, 1024, 2048
H, HD = 16, 128
FF = 8192
EPS = 1e-5
CH = 256              # q-chunk rows
DB = D // 128         # 16
SB = S // 128         # 8
FFB = FF // 128       # 64
CHUNKS = [[0, 3], [1, 2]]  # global q-chunk ids per half

# quantization scales (all powers of two; dequants are compile-time consts)
SW = 256.0            # weight scale for q,k,v,o,gate,up (rms ~0.022 -> ~5.7)
SWD = 512.0           # weight scale for down (rms ~0.011 -> ~5.7)
SX = 8.0              # h1/h2 activation scale (rms 1)
SXO = 16.0            # oT activation scale (o rms ~0.14)
SXH = 8.0             # had activation scale (rms ~0.6, max ~16 -> 128)
ISQ = float(1.0 / np.sqrt(HD))

_CACHE = {}


def _dr(nc, psum, w8, x8h, x8l, kts, msl, start=True, stop=True, wsl=None,
        skip_xl=False):
    """2/3-pass hi/lo DoubleRow GEMM accumulation into psum.

    w8:   [128, 2(hi/lo), KT, m] fp8 weight tile
    x8h/x8l: moving fp8 tensors, sliced as x[:, 2t:2t+2, msl]
    kts:  list of k-tile indices to process (must be even count, paired)
    skip_xl: drop the Wh@xl pass (activation-residual correction) — cheaper
    but adds ~1.1% rel error to this GEMM's output.

    NOTE: start=True zeroes the WHOLE psum bank (lazily, applied at each
    region's first write) — callers accumulating several sub-regions of one
    bank must pass start=True only for the very first sub-region.
    """
    if wsl is None:
        wsl = slice(None)
    passes = [(0, x8h), (1, x8h)] if skip_xl else [(0, x8h), (1, x8h), (0, x8l)]
    np_ = len(passes)
    for pi, (wi, xm) in enumerate(passes):
        for ti in range(0, len(kts), 2):
            t0 = kts[ti]
            first = (pi == 0 and ti == 0)
            last = (pi == np_ - 1 and ti == len(kts) - 2)
            nc.tensor.matmul(psum, w8[:, wi, t0:t0 + 2, wsl],
                             xm[:, t0:t0 + 2, msl],
                             start=(start and first), stop=(stop and last),
                             perf_mode=PM.DoubleRow,
                             skip_group_check=not (start and first))


def _finish_oproj(nc, avps, identH, x1, x_re, n2stats, db, xp, xoT):
    """PSUM o-proj tile -> dequant fp16 -> transpose -> +x -> x1; norm2 stats."""
    dsl = slice(db * 128, (db + 1) * 128)
    nc.scalar.mul(xoT[:], xp[:], 1.0 / (SW * SXO))
    tpf = avps.tile([128, 2, 256], F32, tag="op")
    tp = tpf[:, 0:1, :].bitcast(F16).rearrange("p a (j m) -> p (a j) m", m=128)
    for j in range(4):
        nc.tensor.transpose(tp[:, j, :], xoT[:, j * 128:(j + 1) * 128], identH[:])
    for tg in range(4):
        nc.vector.tensor_tensor(out=x1[:, tg, dsl], in0=tp[:, tg, :],
                                in1=x_re[tg][:, dsl], op=AL.add)
        nc.vector.bn_stats(out=n2stats[:, tg, db, :], in_=x1[:, tg, dsl])


def _finish_down(nc, tps2, identH, x1, outp, ap, db, dp, x2c, split=False):
    """PSUM down tile -> dequant fp16 -> transpose -> +x1 -> out DMA."""
    dsl = slice(db * 128, (db + 1) * 128)
    odb = outp.tile([128, 4, 128], F32, tag="odb")
    outv = ap["out"][db].rearrange("(g p) m -> p g m", p=128)
    for half in ([0] if not split else [0, 1]):
        hw = 4 if not split else 2
        gsl = slice(half * hw, half * hw + hw)
        csl = slice(half * hw * 128, (half * hw + hw) * 128)
        nc.scalar.mul(x2c[:, csl], dp[:, csl], 1.0 / (SWD * SXH))
        tp = tps2.tile([128, hw, 128], F16, tag="tp5")
        for j in range(hw):
            g = half * hw + j
            nc.tensor.transpose(tp[:, j, :], x2c[:, g * 128:(g + 1) * 128], identH[:])
        nc.vector.tensor_tensor(out=odb[:, gsl, :], in0=tp[:],
                                in1=x1[:, gsl, dsl], op=AL.add)
        nc.sync.dma_start(out=outv[:, gsl, :], in_=odb[:, gsl, :])


def _emit(nc, tc, ap, half):
    chunks = CHUNKS[half]
    own_rb = [rb for qc in chunks for rb in (2 * qc, 2 * qc + 1)]

    ctx = ExitStack()
    with ctx:
        persist = ctx.enter_context(tc.tile_pool(name="persist", bufs=1))

        eps_t = persist.tile([128, 1], F32)
        nc.vector.memset(eps_t, EPS / (SX * SX))
        identH = persist.tile([128, 128], F16)
        rotT = persist.tile([128, 128], F16)
        cosT = persist.tile([HD, S], F16)     # pre-scaled by 1/(SW*SX)
        sinT = persist.tile([HD, S], F16)
        maskT = persist.tile([128, 2, 2, CH], F32)

        gbig = ctx.enter_context(tc.tile_pool(name="gbig", bufs=1))
        x1 = gbig.tile([128, 4, D], F16)         # 2MB fp16 (own 512 rows)

        n2stats = gbig.tile([128, 4, DB, 6], F32)
        mw = ctx.enter_context(tc.tile_pool(name="mw", bufs=4))

        # ---- head loop pools (weight pool opened early for prefetch) ----
        hlA = ExitStack()
        wpool = hlA.enter_context(tc.tile_pool(name="wpool", bufs=3))
        small = hlA.enter_context(tc.tile_pool(name="small", bufs=2))
        kqps = hlA.enter_context(tc.tile_pool(name="kqps", bufs=4, space="PSUM"))
        avps = hlA.enter_context(tc.tile_pool(name="avps", bufs=1, space="PSUM"))
        opool = hlA.enter_context(tc.tile_pool(name="opool", bufs=1))
        oTh = opool.tile([128, H, 512], F8)      # 1MB (16*o, hi)
        oTl = opool.tile([128, H, 512], F8)      # 1MB (16*o, lo)
        wop = hlA.enter_context(tc.tile_pool(name="wop", bufs=3))
        xop = hlA.enter_context(tc.tile_pool(name="xop", bufs=4))
        wo_tiles = {}

        def wo_dma(db):
            wo_t = wop.tile([128, 2, H, 128], F8, tag="wo")
            nc.sync.dma_start(
                out=wo_t,
                in_=ap["wo"][db].rearrange("p (two hh m) -> p two hh m",
                                           two=2, m=128))
            wo_tiles[db] = wo_t

        gu_tiles = {}

        def gu_dma(fb):
            wg_t = mw.tile([128, 2, DB, 128], F8, tag="wg")
            wu_t = mw.tile([128, 2, DB, 128], F8, tag="wu")
            nc.sync.dma_start(out=wg_t, in_=ap["wg"][fb].rearrange(
                "p (two db m) -> p two db m", two=2, m=128))
            nc.sync.dma_start(out=wu_t, in_=ap["wu"][fb].rearrange(
                "p (two db m) -> p two db m", two=2, m=128))
            gu_tiles[fb] = (wg_t, wu_t)

        hlB = ExitStack()
        vps = hlB.enter_context(tc.tile_pool(name="vps", bufs=1, space="PSUM"))
        abig = hlB.enter_context(tc.tile_pool(name="abig", bufs=1))
        h1Th = abig.tile([128, DB, S], F8)       # 2MB (8*h1, hi)
        h1Tl = abig.tile([128, DB, S], F8)       # 2MB (8*h1, lo)

        wtiles = {}

        def wdma(h):
            wk_t = wpool.tile([128, 2, DB, 128], F8, tag="wk")
            wv_t = wpool.tile([128, 2, DB, 128], F8, tag="wv")
            wq_t = wpool.tile([128, 2, DB, 128], F8, tag="wq")
            for t, nm in ((wk_t, "wk"), (wv_t, "wv"), (wq_t, "wq")):
                nc.sync.dma_start(out=t, in_=ap[nm][h].rearrange(
                    "p (two db m) -> p two db m", two=2, m=128))
            wtiles[h] = (wk_t, wv_t, wq_t)

        def _kmm(g, wk_t):
            ssl = slice(g * 512, (g + 1) * 512)
            kp = kqps.tile([128, 512], F32, tag="kp")
            _dr(nc, kp[:], wk_t, h1Th, h1Tl, list(range(DB)), ssl,
                start=True, stop=False, skip_xl=True)
            tsin = small.tile([128, 512], F16, tag="tsin")
            nc.vector.tensor_tensor(out=tsin[:], in0=kp[:], in1=sinT[:, ssl], op=AL.mult)
            nc.vector.tensor_tensor(out=kp[:], in0=kp[:], in1=cosT[:, ssl], op=AL.mult)
            return kp, tsin

        def _vmm(vb, wv_t):
            # tokens stationary, weights moving -> natural [token, hd] layout
            vp = vps.tile([128, 4, 128], F32, tag="vp")
            for j in range(4):
                blk = vb * 4 + j
                bsl = slice(blk * 128, (blk + 1) * 128)
                for pi, (wi, xm) in enumerate([(0, h1Th), (1, h1Th), (0, h1Tl)]):
                    for t0 in range(0, DB, 2):
                        nc.tensor.matmul(
                            vp[:, j, :], xm[:, t0:t0 + 2, bsl],
                            wv_t[:, wi, t0:t0 + 2, :],
                            start=(pi == 0 and t0 == 0),
                            stop=(pi == 2 and t0 == DB - 2),
                            perf_mode=PM.DoubleRow, skip_group_check=True)
            return vp

        # ---- phase A: rmsnorm1 -> h1 (fp16, 8x scale) -> transpose ->
        #      split into h1Th/h1Tl fp8 ----
        pres = {0: {}, 1: {}, 2: {}}
        nc.sync.dma_start(out=identH, in_=ap["identH"])
        with tc.tile_pool(name="stA", bufs=2) as stA, \
             tc.tile_pool(name="stAx", bufs=1) as stAx, \
             tc.tile_pool(name="stAps", bufs=2, space="PSUM") as stAps:
            xown = {}
            for rb in [0, 1, 2, 3, "pre", 4, 5, "pre2", 6, 7]:
                if rb == "pre":
                    # head-0/1/2 partial work on tokens 0-511 fills the PE
                    # while the rest of phase A streams through DVE/Act/Pool
                    for hh in (0, 1):
                        pres[hh]["w"] = wtiles.pop(hh)
                        pres[hh]["kp0"], pres[hh]["tsin0"] = \
                            _kmm(0, pres[hh]["w"][0])
                    pres[0]["vp0"] = _vmm(0, pres[0]["w"][1])
                    continue
                if rb == "pre2":
                    pres[2]["w"] = wtiles.pop(2)
                    pres[2]["kp0"], pres[2]["tsin0"] = _kmm(0, pres[2]["w"][0])
                    continue
                if rb in own_rb:
                    x_t = xop.tile([128, D], F16, tag="xot")
                    xown[own_rb.index(rb)] = x_t
                else:
                    x_t = stA.tile([128, D], F16, tag="x_t")
                nc.sync.dma_start(out=x_t, in_=ap["xbh"][rb * 128:(rb + 1) * 128, :])
                xsq = stAx.tile([128, D], F16, tag="xsq")
                acc = stA.tile([128, 1], F32, tag="acc")
                if rb == 1:
                    wdma(0)
                    nc.sync.dma_start(out=rotT, in_=ap["rotT"])
                    nc.sync.dma_start(out=cosT, in_=ap["cosT"])
                    nc.sync.dma_start(out=sinT, in_=ap["sinT"])
                elif rb == 2:
                    wdma(1)
                elif rb == 3:
                    nc.sync.dma_start(out=maskT,
                                      in_=ap["maskT"].rearrange("c k p q -> p c k q"))
                    wdma(2)
                nc.scalar.activation(out=xsq[:], in_=x_t[:], func=AF.Square,
                                     accum_out=acc[:])
                # sd = sqrt(mean(x^2)+eps)/SX ; rstd = SX/sd scale folded in
                sd = stA.tile([128, 1], F32, tag="sdA")
                nc.scalar.activation(out=sd[:], in_=acc[:], func=AF.Sqrt,
                                     scale=float(1.0 / (D * SX * SX)), bias=eps_t[:])
                rstd = stA.tile([128, 1], F32, tag="rstdA")
                nc.vector.reciprocal(out=rstd[:], in_=sd[:])
                h1 = stA.tile([128, D], F16, tag="h1")
                for hk in range(2):
                    hsl = slice(hk * 1024, (hk + 1) * 1024)
                    if hk == 0:
                        nc.scalar.mul(h1[:, hsl], x_t[:, hsl], rstd[:])
                    else:
                        nc.vector.tensor_scalar_mul(h1[:, hsl], x_t[:, hsl], rstd[:])
                    for pk in range(2 * hk, 2 * hk + 2):
                        tp = stAps.tile([128, 4, 128], F16, tag="tpA")
                        for j in range(4):
                            nc.tensor.transpose(
                                tp[:, j, :],
                                h1[:, (4 * pk + j) * 128:(4 * pk + j + 1) * 128],
                                identH[:])
                        rsl = slice(rb * 128, (rb + 1) * 128)
                        dsth = h1Th[:, 4 * pk:4 * pk + 4, rsl]
                        nc.scalar.copy(dsth, tp[:])
                        nc.vector.tensor_tensor(
                            out=h1Tl[:, 4 * pk:4 * pk + 4, rsl],
                            in0=tp[:], in1=dsth, op=AL.subtract)

        kpool = hlB.enter_context(tc.tile_pool(name="kpool", bufs=2))
        ppool = hlB.enter_context(tc.tile_pool(name="ppool", bufs=2))
        sps = hlB.enter_context(tc.tile_pool(name="sps", bufs=2, space="PSUM"))

        hstate = {}

        def _own_msl(qc):
            return slice(qc * CH, (qc + 1) * CH)

        def proj(h, pre=None, mid=None):
            if pre is None:
                wk_t, wv_t, wq_t = wtiles.pop(h)
            else:
                wk_t, wv_t, wq_t = pre["w"]

            kT_h = kpool.tile([128, S], F16, tag="kT")
            v_nat = kpool.tile([128, SB, 128], F16, tag="v_nat")
            qT_h = kpool.tile([128, 512], F16, tag="qT")

            def krot(g, kp, tsin):
                ssl = slice(g * 512, (g + 1) * 512)
                nc.tensor.matmul(kp[:], rotT[:], tsin[:],
                                 start=False, stop=True, skip_group_check=True)
                nc.scalar.copy(kT_h[:, ssl], kp[:])

            if pre is None:
                kp0, tsin0 = _kmm(0, wk_t)
                vp0 = _vmm(0, wv_t)
            else:
                kp0, tsin0 = pre["kp0"], pre["tsin0"]
                vp0 = pre.get("vp0")
                if vp0 is None:
                    vp0 = _vmm(0, wv_t)
            kp1, tsin1 = _kmm(1, wk_t)
            if mid is not None:
                mid()
            krot(0, kp0, tsin0)
            nc.scalar.mul(v_nat[:, 0:4, :], vp0[:], float(SXO / (SW * SX)))

            # q matmuls (own chunks), DoubleRow per chunk
            qp = kqps.tile([128, 512], F32, tag="kp")
            qpv = qp[:].rearrange("p (a c) -> p a c", c=CH)
            for ci, qc in enumerate(chunks):
                # one spanning accumulation group: only the very first matmul
                # carries start=True (bank-wide lazy zero covers chunk 1)
                _dr(nc, qpv[:, ci, :], wq_t, h1Th, h1Tl, list(range(DB)),
                    _own_msl(qc), start=(ci == 0), stop=False, skip_xl=True)
            krot(1, kp1, tsin1)
            tsin = small.tile([128, 512], F16, tag="tsin")
            tsv = tsin[:].rearrange("p (a c) -> p a c", c=CH)
            for ci, qc in enumerate(chunks):
                osl = _own_msl(qc)
                nc.vector.tensor_tensor(out=tsv[:, ci, :], in0=qpv[:, ci, :],
                                        in1=sinT[:, osl], op=AL.mult)
                nc.vector.tensor_tensor(out=qpv[:, ci, :], in0=qpv[:, ci, :],
                                        in1=cosT[:, osl], op=AL.mult)

            vp1 = _vmm(1, wv_t)
            nc.tensor.matmul(qp[:], rotT[:], tsin[:],
                             start=False, stop=True, skip_group_check=True)
            nc.scalar.copy(qT_h[:], qp[:])
            nc.vector.tensor_scalar_mul(v_nat[:, 4:8, :], vp1[:],
                                        float(SXO / (SW * SX)))
            if h + 3 < H:
                wdma(h + 3)
            if h >= 13:
                wo_dma(h - 13)
            hstate[h] = (kT_h, v_nat, qT_h)

        def attn_scores(h):
            kT_h, v_nat, qT_h = hstate.pop(h)
            pcs = []
            for ci, qc in enumerate(chunks):
                nkb = 2 * qc + 2
                osl = slice(ci * CH, (ci + 1) * CH)
                p_sb = ppool.tile([128, SB, CH], F16, tag="p_sb")
                for kb in range(nkb):
                    sp = sps.tile([128, CH], F32, tag="sp")
                    nc.tensor.matmul(sp[:], kT_h[:, kb * 128:(kb + 1) * 128],
                                     qT_h[:, osl], start=True, stop=True,
                                     skip_group_check=True)
                    j = kb - 2 * qc
                    if j >= 0:
                        nc.vector.scalar_tensor_tensor(
                            out=sp[:], in0=sp[:], scalar=ISQ,
                            in1=maskT[:, ci, j, :], op0=AL.mult, op1=AL.add)
                        nc.scalar.activation(out=p_sb[:, kb, :], in_=sp[:],
                                             func=AF.Exp)
                    else:
                        nc.scalar.activation(out=p_sb[:, kb, :], in_=sp[:],
                                             func=AF.Exp, scale=ISQ)
                den = small.tile([128, CH], F32, tag="den")
                nc.vector.tensor_tensor(out=den[:], in0=p_sb[:, 0, :],
                                        in1=p_sb[:, 1, :], op=AL.add)
                for kb in range(2, nkb):
                    nc.vector.tensor_tensor(out=den[:], in0=den[:],
                                            in1=p_sb[:, kb, :], op=AL.add)
                dall = small.tile([128, CH], F32, tag="dall")
                nc.gpsimd.partition_all_reduce(dall[:], den[:], 128,
                                               bass_isa.ReduceOp.add)
                dbc = small.tile([128, CH], F32, tag="dbc")
                nc.vector.reciprocal(out=dbc[:], in_=dall[:])
                pcs.append((nkb, osl, p_sb, dbc))
            return v_nat, pcs

        def attn_av(h, st):
            v_nat, pcs = st
            op_ = avps.tile([128, 2, CH], F32, tag="op")
            for ci in range(2):
                nkb, osl, p_sb, dbc = pcs[ci]
                for kb in range(nkb):
                    nc.tensor.matmul(op_[:, ci, :], v_nat[:, kb, :], p_sb[:, kb, :],
                                     start=(kb == 0), stop=(kb == nkb - 1),
                                     skip_group_check=True)
                t16 = small.tile([128, CH], F16, tag="t16")
                nc.vector.tensor_tensor(out=t16[:], in0=op_[:, ci, :],
                                        in1=dbc[:], op=AL.mult)
                nc.gpsimd.tensor_copy(oTh[:, h, osl], t16[:])
                nc.vector.tensor_tensor(out=oTl[:, h, osl], in0=t16[:],
                                        in1=oTh[:, h, osl], op=AL.subtract)

        ast = {}

        for h in range(H):
            def mid(hh=h):
                if hh > 0:
                    ast[hh - 1] = attn_scores(hh - 1)
            proj(h, pres.get(h) or None, mid=mid)
            if h > 0:
                attn_av(h - 1, ast.pop(h - 1))
        ast[H - 1] = attn_scores(H - 1)
        attn_av(H - 1, ast.pop(H - 1))
        hlB.close()

        # ---- o-proj + residual -> x1 (SBUF) + incremental norm2 stats ----
        with tc.tile_pool(name="st3", bufs=2) as st3:
            pending = None
            for db in range(DB):
                if db + 3 < DB:
                    wo_dma(db + 3)
                if db in (2, 6, 10, 13):
                    gu_dma({2: 0, 6: 1, 10: 2, 13: 3}[db])
                wo_t = wo_tiles.pop(db)
                xp = kqps.tile([128, 512], F32, tag="kp")
                _dr(nc, xp[:], wo_t, oTh, oTl, list(range(H)), slice(None))
                if pending is not None:
                    _finish_oproj(nc, avps, identH, x1, xown, n2stats, *pending)
                xoT = st3.tile([128, 512], F16, tag="xoT")
                pending = (db, xp, xoT)
            _finish_oproj(nc, avps, identH, x1, xown, n2stats, *pending)
        hlA.close()

        mlp = ctx.enter_context(tc.tile_pool(name="mlp", bufs=1))
        h2Th = mlp.tile([128, DB, 512], F8)      # 1MB (8*h2, hi)
        h2Tl = mlp.tile([128, DB, 512], F8)      # 1MB (8*h2, lo)
        hadh = mlp.tile([128, FFB, 512], F8)     # 4MB (8*had, hi)
        hadl = mlp.tile([128, FFB, 512], F8)     # 4MB (8*had, lo)

        # ---- norm2 finalize -> h2Th/h2Tl (fp8, 8x scale) ----
        with tc.tile_pool(name="st4", bufs=4) as st4, \
             tc.tile_pool(name="nps", bufs=4, space="PSUM") as nps:
            diags = []
            for tg in range(4):
                mv = st4.tile([128, 2], F32, tag="n2mv")
                nc.vector.bn_aggr(out=mv[:], in_=n2stats[:, tg, :, :])
                msq = st4.tile([128, 1], F32, tag="n2msq")
                nc.vector.scalar_tensor_tensor(
                    out=msq[:], in0=mv[:, 0:1], scalar=mv[:, 0:1], in1=mv[:, 1:2],
                    op0=AL.mult, op1=AL.add)
                sd = st4.tile([128, 1], F32, tag="n2sd")
                nc.scalar.activation(out=sd[:], in_=msq[:], func=AF.Sqrt,
                                     scale=float(1.0 / (SX * SX)), bias=eps_t[:])
                rstd = st4.tile([128, 1], F32, tag="n2rstd")
                nc.vector.reciprocal(out=rstd[:], in_=sd[:])
                # diag(SX*rstd) fp16; x1_slice^T @ diag transposes + norm-scales
                diag = st4.tile([128, 128], F16, tag="n2diag")
                nc.vector.tensor_scalar_mul(diag[:], identH[:], rstd[:])
                diags.append(diag)
            # pk-major so h2T k-tiles complete in db order
            for pk in range(4):
                for tg in range(4):
                    tp = nps.tile([128, 4, 128], F32, tag="tpN")
                    for j in range(4):
                        nc.tensor.matmul(
                            tp[:, j, :],
                            x1[:, tg, (4 * pk + j) * 128:(4 * pk + j + 1) * 128],
                            diags[tg][:], start=True, stop=True,
                            skip_group_check=True)
                    tsl = slice(tg * 128, (tg + 1) * 128)
                    dsth = h2Th[:, 4 * pk:4 * pk + 4, tsl]
                    nc.scalar.copy(dsth, tp[:])
                    nc.vector.tensor_tensor(out=h2Tl[:, 4 * pk:4 * pk + 4, tsl],
                                            in0=tp[:], in1=dsth, op=AL.subtract)

        # ---- MLP gate/up -> hadh/hadl ----
        wdp_cm = tc.tile_pool(name="wd", bufs=2)
        wdp = wdp_cm.__enter__()
        wd_tiles = {}

        def wd_dma(db):
            wd_t = wdp.tile([128, 2, FFB, 128], F8, tag="wd")
            nc.sync.dma_start(out=wd_t, in_=ap["wd"][db].rearrange(
                "p (two fb m) -> p two fb m", two=2, m=128))
            wd_tiles[db] = wd_t

        with tc.tile_pool(name="mls", bufs=3) as mls, \
             tc.tile_pool(name="gps", bufs=2, space="PSUM") as gps, \
             tc.tile_pool(name="ups", bufs=2, space="PSUM") as ups:
            for fb in range(FFB):
                if fb + 4 < FFB:
                    gu_dma(fb + 4)
                elif fb == FFB - 4:
                    wd_dma(0)
                elif fb == FFB - 2:
                    wd_dma(1)
                wg_t, wu_t = gu_tiles.pop(fb)
                gp = gps.tile([128, 512], F32, tag="gp")
                up = ups.tile([128, 512], F32, tag="up")
                _dr(nc, gp[:], wg_t, h2Th, h2Tl, list(range(DB)), slice(None))
                _dr(nc, up[:], wu_t, h2Th, h2Tl, list(range(DB)), slice(None))
                sg = mls.tile([128, 512], F16, tag="sg")
                nc.scalar.activation(out=sg[:], in_=gp[:], func=AF.Silu,
                                     scale=float(1.0 / (SW * SX)))
                t16 = mls.tile([128, 512], F16, tag="t16m")
                nc.vector.scalar_tensor_tensor(
                    out=t16[:], in0=up[:], scalar=float(SXH / (SW * SX)),
                    in1=sg[:], op0=AL.mult, op1=AL.mult)
                nc.gpsimd.tensor_copy(hadh[:, fb, :], t16[:])
                nc.vector.tensor_tensor(out=hadl[:, fb, :], in0=t16[:],
                                        in1=hadh[:, fb, :], op=AL.subtract)
        # ---- MLP down + final residual -> out ----
        with tc.tile_pool(name="st5", bufs=2) as st5, \
             tc.tile_pool(name="outp", bufs=2) as outp, \
             tc.tile_pool(name="dps", bufs=2, space="PSUM") as dps, \
             tc.tile_pool(name="tps2", bufs=2, space="PSUM") as tps2:
            pending = None
            for db in range(DB):
                if db + 2 < DB:
                    wd_dma(db + 2)
                wd_t = wd_tiles.pop(db)
                dp = dps.tile([128, 512], F32, tag="dp")
                _dr(nc, dp[:], wd_t, hadh, hadl, list(range(FFB)), slice(None))
                if pending is not None:
                    _finish_down(nc, tps2, identH, x1, outp, ap, *pending)
                x2c = st5.tile([128, 512], F16, tag="x2c")
                pending = (db, dp, x2c)
            _finish_down(nc, tps2, identH, x1, outp, ap, *pending, split=True)
        wdp_cm.__exit__(None, None, None)


def _build(half):
    nc = bacc.Bacc("TRN2", target_bir_lowering=False, debug=False, num_devices=8)
    ap = {}

    def din(name, shape, dt=F8):
        ap[name] = nc.dram_tensor(name, shape, dt, kind="ExternalInput").ap()

    din("xbh", [S, D], F16)
    din("cosT", [HD, S], F16)
    din("sinT", [HD, S], F16)
    din("maskT", [2, 2, 128, CH], F32)
    din("identH", [128, 128], F16)
    din("rotT", [128, 128], F16)
    din("wq", [H, 128, 2 * DB * 128]); din("wk", [H, 128, 2 * DB * 128])
    din("wv", [H, 128, 2 * DB * 128])
    din("wo", [DB, 128, 2 * H * 128])
    din("wg", [FFB, 128, 2 * DB * 128]); din("wu", [FFB, 128, 2 * DB * 128])
    din("wd", [DB, 128, 2 * FFB * 128])
    ap["out"] = nc.dram_tensor("out", [DB, 512, 128], F32, kind="ExternalOutput").ap()

    with tile.TileContext(nc) as tc:
        _emit(nc, tc, ap, half)
    nc.compile()
    return nc


def _q8(a):
    return np.asarray(a, np.float32).astype(ml_dtypes.float8_e4m3)


def _hilo(w, s):
    """[.., K-tiles, m] float32 -> stacked hi/lo fp8 at scale s (axis -3)."""
    wh = _q8(w * s)
    wl = _q8(w * s - wh.astype(np.float32))
    return np.stack([wh, wl], axis=-3)


def _prep(inputs):
    inp = {k: np.asarray(v) for k, v in inputs.items()}
    w1 = inp["norm_weight_1"].astype(np.float32)
    w2 = inp["norm_weight_2"].astype(np.float32)

    def fold(n):
        return (inp[f"w_{n}"].astype(np.float32)
                + inp[f"w_{n}_lora_a"].astype(np.float32)
                @ inp[f"w_{n}_lora_b"].astype(np.float32))

    ident = np.eye(128, dtype=np.float16)
    Rm = np.zeros((128, 128), np.float32)
    for i in range(64):
        Rm[i, i + 64] = -1.0
        Rm[i + 64, i] = 1.0

    def _colmajor8(w, nblk, s):
        # [K, N] -> [nblk, 128, 2*(K/128)*(N/nblk)] fp8 hi/lo
        K, N = w.shape
        r = (w.reshape(K // 128, 128, nblk, N // nblk)
             .transpose(2, 1, 0, 3))          # [nblk, 128, KT, n]
        hl = _hilo(r, s)                      # [nblk, 128, 2, KT, n]
        return np.ascontiguousarray(
            hl.reshape(nblk, 128, 2 * (K // 128) * (N // nblk)))

    wo_f = fold("o")
    wo_r = wo_f.reshape(H, 128, DB, 128).transpose(2, 1, 0, 3)  # [DB,128,H,128]
    wo_hl = np.ascontiguousarray(_hilo(wo_r, SW).reshape(DB, 128, 2 * H * 128))
    wd_f = fold("down")
    wd_r = wd_f.reshape(FFB, 128, DB, 128).transpose(2, 1, 0, 3)
    wd_hl = np.ascontiguousarray(_hilo(wd_r, SWD).reshape(DB, 128, 2 * FFB * 128))

    shared = dict(
        wq=_colmajor8(w1[:, None] * fold("q"), H, SW),
        wk=_colmajor8(w1[:, None] * fold("k"), H, SW),
        wv=_colmajor8(w1[:, None] * fold("v"), H, SW),
        wo=wo_hl,
        wg=_colmajor8(w2[:, None] * fold("gate"), FFB, SW),
        wu=_colmajor8(w2[:, None] * fold("up"), FFB, SW),
        wd=wd_hl,
        identH=ident, rotT=np.ascontiguousarray(Rm.T.astype(np.float16)))

    pos = inp["position_ids"].astype(np.int64)
    cs = float(1.0 / (SW * SX))
    cos_p = inp["cos"].astype(np.float32)[pos] * cs
    sin_p = inp["sin"].astype(np.float32)[pos] * cs
    mask = inp["attention_mask"].astype(np.float32)[0, 0]
    x = inp["x"].astype(np.float32)

    in_maps = []
    for c in range(8):
        b = c // 2
        half = c % 2
        mT = np.zeros((2, 2, 128, CH), np.float32)
        for ci, qc in enumerate(CHUNKS[half]):
            for j in range(2):
                kb = 2 * qc + j
                mT[ci, j] = mask[qc * CH:(qc + 1) * CH, kb * 128:(kb + 1) * 128].T
        m = dict(shared)
        m.update(xbh=np.ascontiguousarray(x[b].astype(np.float16)),
                 cosT=np.ascontiguousarray(cos_p[b].T.astype(np.float16)),
                 sinT=np.ascontiguousarray(sin_p[b].T.astype(np.float16)),
                 maskT=mT)
        in_maps.append(m)
    return in_maps


def kernel(**inputs):
    in_maps = _prep(inputs)
    if "nc" not in _CACHE:
        _CACHE["nc"] = (_build(0), _build(1))
    nc0, nc1 = _CACHE["nc"]

    res0 = run_bass_kernel_spmd(nc0, [in_maps[c] for c in (0, 2, 4, 6)],
                                core_ids=[0, 2, 4, 6])
    res1 = run_bass_kernel_spmd(nc1, [in_maps[c] for c in (1, 3, 5, 7)],
                                core_ids=[1, 3, 5, 7])

    out = np.zeros((B, S, D), np.float32)
    for res, halfi, cores in ((res0, 0, (0, 2, 4, 6)), (res1, 1, (1, 3, 5, 7))):
        for gi, c in enumerate(cores):
            b = c // 2
            r = res.results[gi]["out"]   # [DB, 512, 128]
            for ci, qc in enumerate(CHUNKS[halfi]):
                for db in range(DB):
                    out[b, qc * CH:(qc + 1) * CH, db * 128:(db + 1) * 128] = \
                        r[db, ci * CH:(ci + 1) * CH, :]
    return out


# revision 20
# speedup vs baseline: 1.0601x; 1.0139x over previous
"""Trainium2 Bass kernel v3: single transformer layer (attn + gated MLP, LoRA on
all projections), B=4 S=1024 D=2048 H=16 HD=128 FF=8192, fp32 in/out.

Sharding (8 cores, no collectives): core c -> batch b=c//2, q-row chunks
{0,3} (c%2==0) or {1,2} (c%2==1) of 256 rows each (causally balanced).

vs v2: all seven projection GEMMs run as fp8e4 DoubleRow matmuls (2 k-tiles
per instruction at 0.5 cycles/row) with a 3-pass hi/lo error-compensation
scheme  W@x ~= Wh@xh + Wl@xh + Wh@xl  (each operand split into an fp8 "hi"
part plus an fp8 residual "lo" part at the same power-of-2 scale), which is
0.75x the PE cycles of the fp16 baseline at ~fp16-level accuracy.
Attention (scores/AV), RoPE rotation and all transposes stay fp16; softmax,
norms and residuals stay fp32. Dequantization uses fixed power-of-2 scales
folded into the RoPE tables, the softmax Exp, the Silu and epilogue copies.
"""
import numpy as np
import ml_dtypes
from contextlib import ExitStack

import concourse.bass as bass
import concourse.tile as tile
import concourse.mybir as mybir
from concourse import bacc
import concourse.bass_isa as bass_isa
from concourse.bass_utils import run_bass_kernel_spmd

F32 = mybir.dt.float32
F16 = mybir.dt.float16
F8 = mybir.dt.float8e4
AL = mybir.AluOpType
AF = mybir.ActivationFunctionType
PM = mybir.MatmulPerfMode

B, S, D = 4, 1024, 2048
H, HD = 16, 128
FF = 8192
EPS = 1e-5
CH = 256              # q-chunk rows
DB = D // 128         # 16
SB = S // 128         # 8
FFB = FF // 128       # 64
CHUNKS = [[0, 3], [1, 2]]  # global q-chunk ids per half

# quantization scales (all powers of two; dequants are compile-time consts)
SW = 256.0            # weight scale for q,k,v,o,gate,up (rms ~0.022 -> ~5.7)
SWD = 512.0           # weight scale for down (rms ~0.011 -> ~5.7)
SX = 8.0              # h1/h2 activation scale (rms 1)
SXO = 16.0            # oT activation scale (o rms ~0.14)
SXH = 8.0             # had activation scale (rms ~0.6, max ~16 -> 128)
ISQ = float(1.0 / np.sqrt(HD))

_CACHE = {}


def _dr(nc, psum, w8, x8h, x8l, kts, msl, start=True, stop=True, wsl=None,
        skip_xl=False):
    """2/3-pass hi/lo DoubleRow GEMM accumulation into psum.

    w8:   [128, 2(hi/lo), KT, m] fp8 weight tile
    x8h/x8l: moving fp8 tensors, sliced as x[:, 2t:2t+2, msl]
    kts:  list of k-tile indices to process (must be even count, paired)
    skip_xl: drop the Wh@xl pass (activation-residual correction) — cheaper
    but adds ~1.1% rel error to this GEMM's output.

    NOTE: start=True zeroes the WHOLE psum bank (lazily, applied at each
    region's first write) — callers accumulating several sub-regions of one
    bank must pass start=True only for the very first sub-region.
    """
    if wsl is None:
        wsl = slice(None)
    passes = [(0, x8h), (1, x8h)] if skip_xl else [(0, x8h), (1, x8h), (0, x8l)]
    np_ = len(passes)
    for pi, (wi, xm) in enumerate(passes):
        for ti in range(0, len(kts), 2):
            t0 = kts[ti]
            first = (pi == 0 and ti == 0)
            last = (pi == np_ - 1 and ti == len(kts) - 2)
            nc.tensor.matmul(psum, w8[:, wi, t0:t0 + 2, wsl],
                             xm[:, t0:t0 + 2, msl],
                             start=(start and first), stop=(stop and last),
                             perf_mode=PM.DoubleRow,
                             skip_group_check=not (start and first))


def _finish_oproj(nc, avps, identH, x1, x_re, n2stats, db, xp, xoT):
    """PSUM o-proj tile -> dequant fp16 -> transpose -> +x -> x1; norm2 stats."""
    dsl = slice(db * 128, (db + 1) * 128)
    nc.scalar.mul(xoT[:], xp[:], 1.0 / (SW * SXO))
    tpf = avps.tile([128, 2, 256], F32, tag="op")
    tp = tpf[:, 0:1, :].bitcast(F16).rearrange("p a (j m) -> p (a j) m", m=128)
    for j in range(4):
        nc.tensor.transpose(tp[:, j, :], xoT[:, j * 128:(j + 1) * 128], identH[:])
    for tg in range(4):
        nc.vector.tensor_tensor(out=x1[:, tg, dsl], in0=tp[:, tg, :],
                                in1=x_re[tg][:, dsl], op=AL.add)
        nc.vector.bn_stats(out=n2stats[:, tg, db, :], in_=x1[:, tg, dsl])


def _finish_down(nc, tps2, identH, x1, outp, ap, db, dp, x2c, split=False):
    """PSUM down tile -> dequant fp16 -> transpose -> +x1 -> out DMA."""
    dsl = slice(db * 128, (db + 1) * 128)
    odb = outp.tile([128, 4, 128], F32, tag="odb")
    outv = ap["out"][db].rearrange("(g p) m -> p g m", p=128)
    for half in ([0] if not split else [0, 1]):
        hw = 4 if not split else 2
        gsl = slice(half * hw, half * hw + hw)
        csl = slice(half * hw * 128, (half * hw + hw) * 128)
        nc.scalar.mul(x2c[:, csl], dp[:, csl], 1.0 / (SWD * SXH))
        tp = tps2.tile([128, hw, 128], F16, tag="tp5")
        for j in range(hw):
            g = half * hw + j
            nc.tensor.transpose(tp[:, j, :], x2c[:, g * 128:(g + 1) * 128], identH[:])
        nc.vector.tensor_tensor(out=odb[:, gsl, :], in0=tp[:],
                                in1=x1[:, gsl, dsl], op=AL.add)
        nc.sync.dma_start(out=outv[:, gsl, :], in_=odb[:, gsl, :])


def _emit(nc, tc, ap, half):
    chunks = CHUNKS[half]
    own_rb = [rb for qc in chunks for rb in (2 * qc, 2 * qc + 1)]

    ctx = ExitStack()
    with ctx:
        persist = ctx.enter_context(tc.tile_pool(name="persist", bufs=1))

        eps_t = persist.tile([128, 1], F32)
        nc.vector.memset(eps_t, EPS / (SX * SX))
        identH = persist.tile([128, 128], F16)
        rotT = persist.tile([128, 128], F16)
        cosT = persist.tile([HD, S], F16)     # pre-scaled by 1/(SW*SX)
        sinT = persist.tile([HD, S], F16)
        maskT = persist.tile([128, 2, 2, CH], F32)

        gbig = ctx.enter_context(tc.tile_pool(name="gbig", bufs=1))
        x1 = gbig.tile([128, 4, D], F16)         # 2MB fp16 (own 512 rows)

        n2stats = gbig.tile([128, 4, DB, 6], F32)
        mw = ctx.enter_context(tc.tile_pool(name="mw", bufs=4))

        # ---- head loop pools (weight pool opened early for prefetch) ----
        hlA = ExitStack()
        wpool = hlA.enter_context(tc.tile_pool(name="wpool", bufs=3))
        small = hlA.enter_context(tc.tile_pool(name="small", bufs=2))
        kqps = hlA.enter_context(tc.tile_pool(name="kqps", bufs=4, space="PSUM"))
        avps = hlA.enter_context(tc.tile_pool(name="avps", bufs=1, space="PSUM"))
        opool = hlA.enter_context(tc.tile_pool(name="opool", bufs=1))
        oTh = opool.tile([128, H, 512], F8)      # 1MB (16*o, hi)
        oTl = opool.tile([128, H, 512], F8)      # 1MB (16*o, lo)
        wop = hlA.enter_context(tc.tile_pool(name="wop", bufs=3))
        xop = hlA.enter_context(tc.tile_pool(name="xop", bufs=4))
        wo_tiles = {}

        def wo_dma(db):
            wo_t = wop.tile([128, 2, H, 128], F8, tag="wo")
            nc.sync.dma_start(
                out=wo_t,
                in_=ap["wo"][db].rearrange("p (two hh m) -> p two hh m",
                                           two=2, m=128))
            wo_tiles[db] = wo_t

        gu_tiles = {}

        def gu_dma(fb):
            wg_t = mw.tile([128, 2, DB, 128], F8, tag="wg")
            wu_t = mw.tile([128, 2, DB, 128], F8, tag="wu")
            nc.sync.dma_start(out=wg_t, in_=ap["wg"][fb].rearrange(
                "p (two db m) -> p two db m", two=2, m=128))
            nc.sync.dma_start(out=wu_t, in_=ap["wu"][fb].rearrange(
                "p (two db m) -> p two db m", two=2, m=128))
            gu_tiles[fb] = (wg_t, wu_t)

        hlB = ExitStack()
        vps = hlB.enter_context(tc.tile_pool(name="vps", bufs=1, space="PSUM"))
        abig = hlB.enter_context(tc.tile_pool(name="abig", bufs=1))
        h1Th = abig.tile([128, DB, S], F8)       # 2MB (8*h1, hi)
        h1Tl = abig.tile([128, DB, S], F8)       # 2MB (8*h1, lo)

        wtiles = {}

        def wdma(h):
            wk_t = wpool.tile([128, 2, DB, 128], F8, tag="wk")
            wv_t = wpool.tile([128, 2, DB, 128], F8, tag="wv")
            wq_t = wpool.tile([128, 2, DB, 128], F8, tag="wq")
            for t, nm in ((wk_t, "wk"), (wv_t, "wv"), (wq_t, "wq")):
                nc.sync.dma_start(out=t, in_=ap[nm][h].rearrange(
                    "p (two db m) -> p two db m", two=2, m=128))
            wtiles[h] = (wk_t, wv_t, wq_t)

        def _kmm(g, wk_t):
            ssl = slice(g * 512, (g + 1) * 512)
            kp = kqps.tile([128, 512], F32, tag="kp")
            _dr(nc, kp[:], wk_t, h1Th, h1Tl, list(range(DB)), ssl,
                start=True, stop=False)
            tsin = small.tile([128, 512], F16, tag="tsin")
            nc.vector.tensor_tensor(out=tsin[:], in0=kp[:], in1=sinT[:, ssl], op=AL.mult)
            nc.vector.tensor_tensor(out=kp[:], in0=kp[:], in1=cosT[:, ssl], op=AL.mult)
            return kp, tsin

        def _vmm(vb, wv_t):
            # tokens stationary, weights moving -> natural [token, hd] layout
            vp = vps.tile([128, 4, 128], F32, tag="vp")
            for j in range(4):
                blk = vb * 4 + j
                bsl = slice(blk * 128, (blk + 1) * 128)
                for pi, (wi, xm) in enumerate([(0, h1Th), (1, h1Th), (0, h1Tl)]):
                    for t0 in range(0, DB, 2):
                        nc.tensor.matmul(
                            vp[:, j, :], xm[:, t0:t0 + 2, bsl],
                            wv_t[:, wi, t0:t0 + 2, :],
                            start=(pi == 0 and t0 == 0),
                            stop=(pi == 2 and t0 == DB - 2),
                            perf_mode=PM.DoubleRow, skip_group_check=True)
            return vp

        # ---- phase A: rmsnorm1 -> h1 (fp16, 8x scale) -> transpose ->
        #      split into h1Th/h1Tl fp8 ----
        pres = {0: {}, 1: {}, 2: {}}
        nc.sync.dma_start(out=identH, in_=ap["identH"])
        with tc.tile_pool(name="stA", bufs=2) as stA, \
             tc.tile_pool(name="stAx", bufs=1) as stAx, \
             tc.tile_pool(name="stAps", bufs=2, space="PSUM") as stAps:
            xown = {}
            for rb in [0, 1, 2, 3, "pre", 4, 5, "pre2", 6, 7]:
                if rb == "pre":
                    # head-0/1/2 partial work on tokens 0-511 fills the PE
                    # while the rest of phase A streams through DVE/Act/Pool
                    for hh in (0, 1):
                        pres[hh]["w"] = wtiles.pop(hh)
                        pres[hh]["kp0"], pres[hh]["tsin0"] = \
                            _kmm(0, pres[hh]["w"][0])
                    pres[0]["vp0"] = _vmm(0, pres[0]["w"][1])
                    continue
                if rb == "pre2":
                    pres[2]["w"] = wtiles.pop(2)
                    pres[2]["kp0"], pres[2]["tsin0"] = _kmm(0, pres[2]["w"][0])
                    continue
                if rb in own_rb:
                    x_t = xop.tile([128, D], F16, tag="xot")
                    xown[own_rb.index(rb)] = x_t
                else:
                    x_t = stA.tile([128, D], F16, tag="x_t")
                nc.sync.dma_start(out=x_t, in_=ap["xbh"][rb * 128:(rb + 1) * 128, :])
                xsq = stAx.tile([128, D], F16, tag="xsq")
                acc = stA.tile([128, 1], F32, tag="acc")
                if rb == 1:
                    wdma(0)
                    nc.sync.dma_start(out=rotT, in_=ap["rotT"])
                    nc.sync.dma_start(out=cosT, in_=ap["cosT"])
                    nc.sync.dma_start(out=sinT, in_=ap["sinT"])
                elif rb == 2:
                    wdma(1)
                elif rb == 3:
                    nc.sync.dma_start(out=maskT,
                                      in_=ap["maskT"].rearrange("c k p q -> p c k q"))
                    wdma(2)
                nc.scalar.activation(out=xsq[:], in_=x_t[:], func=AF.Square,
                                     accum_out=acc[:])
                # sd = sqrt(mean(x^2)+eps)/SX ; rstd = SX/sd scale folded in
                sd = stA.tile([128, 1], F32, tag="sdA")
                nc.scalar.activation(out=sd[:], in_=acc[:], func=AF.Sqrt,
                                     scale=float(1.0 / (D * SX * SX)), bias=eps_t[:])
                rstd = stA.tile([128, 1], F32, tag="rstdA")
                nc.vector.reciprocal(out=rstd[:], in_=sd[:])
                h1 = stA.tile([128, D], F16, tag="h1")
                for hk in range(2):
                    hsl = slice(hk * 1024, (hk + 1) * 1024)
                    if hk == 0:
                        nc.scalar.mul(h1[:, hsl], x_t[:, hsl], rstd[:])
                    else:
                        nc.vector.tensor_scalar_mul(h1[:, hsl], x_t[:, hsl], rstd[:])
                    for pk in range(2 * hk, 2 * hk + 2):
                        tp = stAps.tile([128, 4, 128], F16, tag="tpA")
                        for j in range(4):
                            nc.tensor.transpose(
                                tp[:, j, :],
                                h1[:, (4 * pk + j) * 128:(4 * pk + j + 1) * 128],
                                identH[:])
                        rsl = slice(rb * 128, (rb + 1) * 128)
                        dsth = h1Th[:, 4 * pk:4 * pk + 4, rsl]
                        nc.scalar.copy(dsth, tp[:])
                        nc.vector.tensor_tensor(
                            out=h1Tl[:, 4 * pk:4 * pk + 4, rsl],
                            in0=tp[:], in1=dsth, op=AL.subtract)

        kpool = hlB.enter_context(tc.tile_pool(name="kpool", bufs=2))
        ppool = hlB.enter_context(tc.tile_pool(name="ppool", bufs=2))
        sps = hlB.enter_context(tc.tile_pool(name="sps", bufs=2, space="PSUM"))

        hstate = {}

        def _own_msl(qc):
            return slice(qc * CH, (qc + 1) * CH)

        def proj(h, pre=None, mid=None):
            if pre is None:
                wk_t, wv_t, wq_t = wtiles.pop(h)
            else:
                wk_t, wv_t, wq_t = pre["w"]

            kT_h = kpool.tile([128, S], F16, tag="kT")
            v_nat = kpool.tile([128, SB, 128], F16, tag="v_nat")
            qT_h = kpool.tile([128, 512], F16, tag="qT")

            def krot(g, kp, tsin):
                ssl = slice(g * 512, (g + 1) * 512)
                nc.tensor.matmul(kp[:], rotT[:], tsin[:],
                                 start=False, stop=True, skip_group_check=True)
                nc.scalar.copy(kT_h[:, ssl], kp[:])

            if pre is None:
                kp0, tsin0 = _kmm(0, wk_t)
                vp0 = _vmm(0, wv_t)
            else:
                kp0, tsin0 = pre["kp0"], pre["tsin0"]
                vp0 = pre.get("vp0")
                if vp0 is None:
                    vp0 = _vmm(0, wv_t)
            kp1, tsin1 = _kmm(1, wk_t)
            if mid is not None:
                mid()
            krot(0, kp0, tsin0)
            nc.scalar.mul(v_nat[:, 0:4, :], vp0[:], float(SXO / (SW * SX)))

            # q matmuls (own chunks), DoubleRow per chunk
            qp = kqps.tile([128, 512], F32, tag="kp")
            qpv = qp[:].rearrange("p (a c) -> p a c", c=CH)
            for ci, qc in enumerate(chunks):
                # one spanning accumulation group: only the very first matmul
                # carries start=True (bank-wide lazy zero covers chunk 1)
                _dr(nc, qpv[:, ci, :], wq_t, h1Th, h1Tl, list(range(DB)),
                    _own_msl(qc), start=(ci == 0), stop=False)
            krot(1, kp1, tsin1)
            tsin = small.tile([128, 512], F16, tag="tsin")
            tsv = tsin[:].rearrange("p (a c) -> p a c", c=CH)
            for ci, qc in enumerate(chunks):
                osl = _own_msl(qc)
                nc.vector.tensor_tensor(out=tsv[:, ci, :], in0=qpv[:, ci, :],
                                        in1=sinT[:, osl], op=AL.mult)
                nc.vector.tensor_tensor(out=qpv[:, ci, :], in0=qpv[:, ci, :],
                                        in1=cosT[:, osl], op=AL.mult)

            vp1 = _vmm(1, wv_t)
            nc.tensor.matmul(qp[:], rotT[:], tsin[:],
                             start=False, stop=True, skip_group_check=True)
            nc.scalar.copy(qT_h[:], qp[:])
            nc.vector.tensor_scalar_mul(v_nat[:, 4:8, :], vp1[:],
                                        float(SXO / (SW * SX)))
            if h + 3 < H:
                wdma(h + 3)
            if h >= 13:
                wo_dma(h - 13)
            hstate[h] = (kT_h, v_nat, qT_h)

        def attn_scores(h):
            kT_h, v_nat, qT_h = hstate.pop(h)
            pcs = []
            for ci, qc in enumerate(chunks):
                nkb = 2 * qc + 2
                osl = slice(ci * CH, (ci + 1) * CH)
                p_sb = ppool.tile([128, SB, CH], F16, tag="p_sb")
                for kb in range(nkb):
                    sp = sps.tile([128, CH], F32, tag="sp")
                    nc.tensor.matmul(sp[:], kT_h[:, kb * 128:(kb + 1) * 128],
                                     qT_h[:, osl], start=True, stop=True,
                                     skip_group_check=True)
                    j = kb - 2 * qc
                    if j >= 0:
                        nc.vector.scalar_tensor_tensor(
                            out=sp[:], in0=sp[:], scalar=ISQ,
                            in1=maskT[:, ci, j, :], op0=AL.mult, op1=AL.add)
                        nc.scalar.activation(out=p_sb[:, kb, :], in_=sp[:],
                                             func=AF.Exp)
                    else:
                        nc.scalar.activation(out=p_sb[:, kb, :], in_=sp[:],
                                             func=AF.Exp, scale=ISQ)
                den = small.tile([128, CH], F32, tag="den")
                nc.vector.tensor_tensor(out=den[:], in0=p_sb[:, 0, :],
                                        in1=p_sb[:, 1, :], op=AL.add)
                for kb in range(2, nkb):
                    nc.vector.tensor_tensor(out=den[:], in0=den[:],
                                            in1=p_sb[:, kb, :], op=AL.add)
                dall = small.tile([128, CH], F32, tag="dall")
                nc.gpsimd.partition_all_reduce(dall[:], den[:], 128,
                                               bass_isa.ReduceOp.add)
                dbc = small.tile([128, CH], F32, tag="dbc")
                nc.vector.reciprocal(out=dbc[:], in_=dall[:])
                pcs.append((nkb, osl, p_sb, dbc))
            return v_nat, pcs

        def attn_av(h, st):
            v_nat, pcs = st
            op_ = avps.tile([128, 2, CH], F32, tag="op")
            for ci in range(2):
                nkb, osl, p_sb, dbc = pcs[ci]
                for kb in range(nkb):
                    nc.tensor.matmul(op_[:, ci, :], v_nat[:, kb, :], p_sb[:, kb, :],
                                     start=(kb == 0), stop=(kb == nkb - 1),
                                     skip_group_check=True)
                t16 = small.tile([128, CH], F16, tag="t16")
                nc.vector.tensor_tensor(out=t16[:], in0=op_[:, ci, :],
                                        in1=dbc[:], op=AL.mult)
                nc.gpsimd.tensor_copy(oTh[:, h, osl], t16[:])
                nc.vector.tensor_tensor(out=oTl[:, h, osl], in0=t16[:],
                                        in1=oTh[:, h, osl], op=AL.subtract)

        ast = {}

        for h in range(H):
            def mid(hh=h):
                if hh > 0:
                    ast[hh - 1] = attn_scores(hh - 1)
            proj(h, pres.get(h) or None, mid=mid)
            if h > 0:
                attn_av(h - 1, ast.pop(h - 1))
        ast[H - 1] = attn_scores(H - 1)
        attn_av(H - 1, ast.pop(H - 1))
        hlB.close()

        # ---- o-proj + residual -> x1 (SBUF) + incremental norm2 stats ----
        with tc.tile_pool(name="st3", bufs=2) as st3:
            pending = None
            for db in range(DB):
                if db + 3 < DB:
                    wo_dma(db + 3)
                if db in (2, 6, 10, 13):
                    gu_dma({2: 0, 6: 1, 10: 2, 13: 3}[db])
                wo_t = wo_tiles.pop(db)
                xp = kqps.tile([128, 512], F32, tag="kp")
                _dr(nc, xp[:], wo_t, oTh, oTl, list(range(H)), slice(None))
                if pending is not None:
                    _finish_oproj(nc, avps, identH, x1, xown, n2stats, *pending)
                xoT = st3.tile([128, 512], F16, tag="xoT")
                pending = (db, xp, xoT)
            _finish_oproj(nc, avps, identH, x1, xown, n2stats, *pending)
        hlA.close()

        mlp = ctx.enter_context(tc.tile_pool(name="mlp", bufs=1))
        h2Th = mlp.tile([128, DB, 512], F8)      # 1MB (8*h2, hi)
        h2Tl = mlp.tile([128, DB, 512], F8)      # 1MB (8*h2, lo)
        hadh = mlp.tile([128, FFB, 512], F8)     # 4MB (8*had, hi)

        # ---- norm2 finalize -> h2Th/h2Tl (fp8, 8x scale) ----
        with tc.tile_pool(name="st4", bufs=4) as st4, \
             tc.tile_pool(name="nps", bufs=4, space="PSUM") as nps:
            diags = []
            for tg in range(4):
                mv = st4.tile([128, 2], F32, tag="n2mv")
                nc.vector.bn_aggr(out=mv[:], in_=n2stats[:, tg, :, :])
                msq = st4.tile([128, 1], F32, tag="n2msq")
                nc.vector.scalar_tensor_tensor(
                    out=msq[:], in0=mv[:, 0:1], scalar=mv[:, 0:1], in1=mv[:, 1:2],
                    op0=AL.mult, op1=AL.add)
                sd = st4.tile([128, 1], F32, tag="n2sd")
                nc.scalar.activation(out=sd[:], in_=msq[:], func=AF.Sqrt,
                                     scale=float(1.0 / (SX * SX)), bias=eps_t[:])
                rstd = st4.tile([128, 1], F32, tag="n2rstd")
                nc.vector.reciprocal(out=rstd[:], in_=sd[:])
                # diag(SX*rstd) fp16; x1_slice^T @ diag transposes + norm-scales
                diag = st4.tile([128, 128], F16, tag="n2diag")
                nc.vector.tensor_scalar_mul(diag[:], identH[:], rstd[:])
                diags.append(diag)
            # pk-major so h2T k-tiles complete in db order
            for pk in range(4):
                for tg in range(4):
                    tp = nps.tile([128, 4, 128], F32, tag="tpN")
                    for j in range(4):
                        nc.tensor.matmul(
                            tp[:, j, :],
                            x1[:, tg, (4 * pk + j) * 128:(4 * pk + j + 1) * 128],
                            diags[tg][:], start=True, stop=True,
                            skip_group_check=True)
                    tsl = slice(tg * 128, (tg + 1) * 128)
                    dsth = h2Th[:, 4 * pk:4 * pk + 4, tsl]
                    nc.scalar.copy(dsth, tp[:])
                    nc.vector.tensor_tensor(out=h2Tl[:, 4 * pk:4 * pk + 4, tsl],
                                            in0=tp[:], in1=dsth, op=AL.subtract)

        # ---- MLP gate/up -> hadh/hadl ----
        wdp_cm = tc.tile_pool(name="wd", bufs=2)
        wdp = wdp_cm.__enter__()
        wd_tiles = {}

        def wd_dma(db):
            wd_t = wdp.tile([128, 2, FFB, 128], F8, tag="wd")
            nc.sync.dma_start(out=wd_t, in_=ap["wd"][db].rearrange(
                "p (two fb m) -> p two fb m", two=2, m=128))
            wd_tiles[db] = wd_t

        with tc.tile_pool(name="mls", bufs=3) as mls, \
             tc.tile_pool(name="gps", bufs=2, space="PSUM") as gps, \
             tc.tile_pool(name="ups", bufs=2, space="PSUM") as ups:
            for fb in range(FFB):
                if fb + 4 < FFB:
                    gu_dma(fb + 4)
                elif fb == FFB - 4:
                    wd_dma(0)
                elif fb == FFB - 2:
                    wd_dma(1)
                wg_t, wu_t = gu_tiles.pop(fb)
                gp = gps.tile([128, 512], F32, tag="gp")
                up = ups.tile([128, 512], F32, tag="up")
                _dr(nc, gp[:], wg_t, h2Th, h2Tl, list(range(DB)), slice(None))
                _dr(nc, up[:], wu_t, h2Th, h2Tl, list(range(DB)), slice(None))
                sg = mls.tile([128, 512], F16, tag="sg")
                nc.scalar.activation(out=sg[:], in_=gp[:], func=AF.Silu,
                                     scale=float(1.0 / (SW * SX)))
                t16 = mls.tile([128, 512], F16, tag="t16m")
                nc.vector.scalar_tensor_tensor(
                    out=t16[:], in0=up[:], scalar=float(SXH / (SW * SX)),
                    in1=sg[:], op0=AL.mult, op1=AL.mult)
                nc.gpsimd.tensor_copy(hadh[:, fb, :], t16[:])
        # ---- MLP down + final residual -> out ----
        with tc.tile_pool(name="st5", bufs=2) as st5, \
             tc.tile_pool(name="outp", bufs=2) as outp, \
             tc.tile_pool(name="dps", bufs=2, space="PSUM") as dps, \
             tc.tile_pool(name="tps2", bufs=2, space="PSUM") as tps2:
            pending = None
            for db in range(DB):
                if db + 2 < DB:
                    wd_dma(db + 2)
                wd_t = wd_tiles.pop(db)
                dp = dps.tile([128, 512], F32, tag="dp")
                _dr(nc, dp[:], wd_t, hadh, None, list(range(FFB)), slice(None),
                    skip_xl=True)
                if pending is not None:
                    _finish_down(nc, tps2, identH, x1, outp, ap, *pending)
                x2c = st5.tile([128, 512], F16, tag="x2c")
                pending = (db, dp, x2c)
            _finish_down(nc, tps2, identH, x1, outp, ap, *pending, split=True)
        wdp_cm.__exit__(None, None, None)


def _build(half):
    nc = bacc.Bacc("TRN2", target_bir_lowering=False, debug=False, num_devices=8)
    ap = {}

    def din(name, shape, dt=F8):
        ap[name] = nc.dram_tensor(name, shape, dt, kind="ExternalInput").ap()

    din("xbh", [S, D], F16)
    din("cosT", [HD, S], F16)
    din("sinT", [HD, S], F16)
    din("maskT", [2, 2, 128, CH], F32)
    din("identH", [128, 128], F16)
    din("rotT", [128, 128], F16)
    din("wq", [H, 128, 2 * DB * 128]); din("wk", [H, 128, 2 * DB * 128])
    din("wv", [H, 128, 2 * DB * 128])
    din("wo", [DB, 128, 2 * H * 128])
    din("wg", [FFB, 128, 2 * DB * 128]); din("wu", [FFB, 128, 2 * DB * 128])
    din("wd", [DB, 128, 2 * FFB * 128])
    ap["out"] = nc.dram_tensor("out", [DB, 512, 128], F32, kind="ExternalOutput").ap()

    with tile.TileContext(nc) as tc:
        _emit(nc, tc, ap, half)
    nc.compile()
    return nc


def _q8(a):
    return np.asarray(a, np.float32).astype(ml_dtypes.float8_e4m3)


def _hilo(w, s):
    """[.., K-tiles, m] float32 -> stacked hi/lo fp8 at scale s (axis -3)."""
    wh = _q8(w * s)
    wl = _q8(w * s - wh.astype(np.float32))
    return np.stack([wh, wl], axis=-3)


def _prep(inputs):
    inp = {k: np.asarray(v) for k, v in inputs.items()}
    w1 = inp["norm_weight_1"].astype(np.float32)
    w2 = inp["norm_weight_2"].astype(np.float32)

    def fold(n):
        return (inp[f"w_{n}"].astype(np.float32)
                + inp[f"w_{n}_lora_a"].astype(np.float32)
                @ inp[f"w_{n}_lora_b"].astype(np.float32))

    ident = np.eye(128, dtype=np.float16)
    Rm = np.zeros((128, 128), np.float32)
    for i in range(64):
        Rm[i, i + 64] = -1.0
        Rm[i + 64, i] = 1.0

    def _colmajor8(w, nblk, s):
        # [K, N] -> [nblk, 128, 2*(K/128)*(N/nblk)] fp8 hi/lo
        K, N = w.shape
        r = (w.reshape(K // 128, 128, nblk, N // nblk)
             .transpose(2, 1, 0, 3))          # [nblk, 128, KT, n]
        hl = _hilo(r, s)                      # [nblk, 128, 2, KT, n]
        return np.ascontiguousarray(
            hl.reshape(nblk, 128, 2 * (K // 128) * (N // nblk)))

    wo_f = fold("o")
    wo_r = wo_f.reshape(H, 128, DB, 128).transpose(2, 1, 0, 3)  # [DB,128,H,128]
    wo_hl = np.ascontiguousarray(_hilo(wo_r, SW).reshape(DB, 128, 2 * H * 128))
    wd_f = fold("down")
    wd_r = wd_f.reshape(FFB, 128, DB, 128).transpose(2, 1, 0, 3)
    wd_hl = np.ascontiguousarray(_hilo(wd_r, SWD).reshape(DB, 128, 2 * FFB * 128))

    shared = dict(
        wq=_colmajor8(w1[:, None] * fold("q"), H, SW),
        wk=_colmajor8(w1[:, None] * fold("k"), H, SW),
        wv=_colmajor8(w1[:, None] * fold("v"), H, SW),
        wo=wo_hl,
        wg=_colmajor8(w2[:, None] * fold("gate"), FFB, SW),
        wu=_colmajor8(w2[:, None] * fold("up"), FFB, SW),
        wd=wd_hl,
        identH=ident, rotT=np.ascontiguousarray(Rm.T.astype(np.float16)))

    pos = inp["position_ids"].astype(np.int64)
    cs = float(1.0 / (SW * SX))
    cos_p = inp["cos"].astype(np.float32)[pos] * cs
    sin_p = inp["sin"].astype(np.float32)[pos] * cs
    mask = inp["attention_mask"].astype(np.float32)[0, 0]
    x = inp["x"].astype(np.float32)

    in_maps = []
    for c in range(8):
        b = c // 2
        half = c % 2
        mT = np.zeros((2, 2, 128, CH), np.float32)
        for ci, qc in enumerate(CHUNKS[half]):
            for j in range(2):
                kb = 2 * qc + j
                mT[ci, j] = mask[qc * CH:(qc + 1) * CH, kb * 128:(kb + 1) * 128].T
        m = dict(shared)
        m.update(xbh=np.ascontiguousarray(x[b].astype(np.float16)),
                 cosT=np.ascontiguousarray(cos_p[b].T.astype(np.float16)),
                 sinT=np.ascontiguousarray(sin_p[b].T.astype(np.float16)),
                 maskT=mT)
        in_maps.append(m)
    return in_maps


def kernel(**inputs):
    in_maps = _prep(inputs)
    if "nc" not in _CACHE:
        _CACHE["nc"] = (_build(0), _build(1))
    nc0, nc1 = _CACHE["nc"]

    res0 = run_bass_kernel_spmd(nc0, [in_maps[c] for c in (0, 2, 4, 6)],
                                core_ids=[0, 2, 4, 6])
    res1 = run_bass_kernel_spmd(nc1, [in_maps[c] for c in (1, 3, 5, 7)],
                                core_ids=[1, 3, 5, 7])

    out = np.zeros((B, S, D), np.float32)
    for res, halfi, cores in ((res0, 0, (0, 2, 4, 6)), (res1, 1, (1, 3, 5, 7))):
        for gi, c in enumerate(cores):
            b = c // 2
            r = res.results[gi]["out"]   # [DB, 512, 128]
            for ci, qc in enumerate(CHUNKS[halfi]):
                for db in range(DB):
                    out[b, qc * CH:(qc + 1) * CH, db * 128:(db + 1) * 128] = \
                        r[db, ci * CH:(ci + 1) * CH, :]
    return out


# revision 21
# speedup vs baseline: 1.0630x; 1.0027x over previous
"""Trainium2 Bass kernel v3: single transformer layer (attn + gated MLP, LoRA on
all projections), B=4 S=1024 D=2048 H=16 HD=128 FF=8192, fp32 in/out.

Sharding (8 cores, no collectives): core c -> batch b=c//2, q-row chunks
{0,3} (c%2==0) or {1,2} (c%2==1) of 256 rows each (causally balanced).

vs v2: all seven projection GEMMs run as fp8e4 DoubleRow matmuls (2 k-tiles
per instruction at 0.5 cycles/row) with a 3-pass hi/lo error-compensation
scheme  W@x ~= Wh@xh + Wl@xh + Wh@xl  (each operand split into an fp8 "hi"
part plus an fp8 residual "lo" part at the same power-of-2 scale), which is
0.75x the PE cycles of the fp16 baseline at ~fp16-level accuracy.
Attention (scores/AV), RoPE rotation and all transposes stay fp16; softmax,
norms and residuals stay fp32. Dequantization uses fixed power-of-2 scales
folded into the RoPE tables, the softmax Exp, the Silu and epilogue copies.
"""
import numpy as np
import ml_dtypes
from contextlib import ExitStack

import concourse.bass as bass
import concourse.tile as tile
import concourse.mybir as mybir
from concourse import bacc
import concourse.bass_isa as bass_isa
from concourse.bass_utils import run_bass_kernel_spmd

F32 = mybir.dt.float32
F16 = mybir.dt.float16
F8 = mybir.dt.float8e4
AL = mybir.AluOpType
AF = mybir.ActivationFunctionType
PM = mybir.MatmulPerfMode

B, S, D = 4, 1024, 2048
H, HD = 16, 128
FF = 8192
EPS = 1e-5
CH = 256              # q-chunk rows
DB = D // 128         # 16
SB = S // 128         # 8
FFB = FF // 128       # 64
CHUNKS = [[0, 3], [1, 2]]  # global q-chunk ids per half

# quantization scales (all powers of two; dequants are compile-time consts)
SW = 256.0            # weight scale for q,k,v,o,gate,up (rms ~0.022 -> ~5.7)
SWD = 512.0           # weight scale for down (rms ~0.011 -> ~5.7)
SX = 8.0              # h1/h2 activation scale (rms 1)
SXO = 16.0            # oT activation scale (o rms ~0.14)
SXH = 8.0             # had activation scale (rms ~0.6, max ~16 -> 128)
ISQ = float(1.0 / np.sqrt(HD))

_CACHE = {}


def _dr(nc, psum, w8, x8h, x8l, kts, msl, start=True, stop=True, wsl=None,
        skip_xl=False):
    """2/3-pass hi/lo DoubleRow GEMM accumulation into psum.

    w8:   [128, 2(hi/lo), KT, m] fp8 weight tile
    x8h/x8l: moving fp8 tensors, sliced as x[:, 2t:2t+2, msl]
    kts:  list of k-tile indices to process (must be even count, paired)
    skip_xl: drop the Wh@xl pass (activation-residual correction) — cheaper
    but adds ~1.1% rel error to this GEMM's output.

    NOTE: start=True zeroes the WHOLE psum bank (lazily, applied at each
    region's first write) — callers accumulating several sub-regions of one
    bank must pass start=True only for the very first sub-region.
    """
    if wsl is None:
        wsl = slice(None)
    passes = [(0, x8h), (1, x8h)] if skip_xl else [(0, x8h), (1, x8h), (0, x8l)]
    np_ = len(passes)
    for pi, (wi, xm) in enumerate(passes):
        for ti in range(0, len(kts), 2):
            t0 = kts[ti]
            first = (pi == 0 and ti == 0)
            last = (pi == np_ - 1 and ti == len(kts) - 2)
            nc.tensor.matmul(psum, w8[:, wi, t0:t0 + 2, wsl],
                             xm[:, t0:t0 + 2, msl],
                             start=(start and first), stop=(stop and last),
                             perf_mode=PM.DoubleRow,
                             skip_group_check=not (start and first))


def _finish_oproj(nc, avps, identH, x1, x_re, n2stats, db, xp, xoT):
    """PSUM o-proj tile -> dequant fp16 -> transpose -> +x -> x1; norm2 stats."""
    dsl = slice(db * 128, (db + 1) * 128)
    nc.scalar.mul(xoT[:], xp[:], 1.0 / (SW * SXO))
    tpf = avps.tile([128, 2, 256], F32, tag="op")
    tp = tpf[:, 0:1, :].bitcast(F16).rearrange("p a (j m) -> p (a j) m", m=128)
    for j in range(4):
        nc.tensor.transpose(tp[:, j, :], xoT[:, j * 128:(j + 1) * 128], identH[:])
    for tg in range(4):
        nc.vector.tensor_tensor(out=x1[:, tg, dsl], in0=tp[:, tg, :],
                                in1=x_re[tg][:, dsl], op=AL.add)
        nc.vector.bn_stats(out=n2stats[:, tg, db, :], in_=x1[:, tg, dsl])


def _finish_down(nc, tps2, identH, x1, outp, ap, db, dp, x2c, split=False):
    """PSUM down tile -> dequant fp16 -> transpose -> +x1 -> out DMA."""
    dsl = slice(db * 128, (db + 1) * 128)
    odb = outp.tile([128, 4, 128], F32, tag="odb")
    outv = ap["out"][db].rearrange("(g p) m -> p g m", p=128)
    for half in ([0] if not split else [0, 1]):
        hw = 4 if not split else 2
        gsl = slice(half * hw, half * hw + hw)
        csl = slice(half * hw * 128, (half * hw + hw) * 128)
        nc.scalar.mul(x2c[:, csl], dp[:, csl], 1.0 / (SWD * SXH))
        tp = tps2.tile([128, hw, 128], F16, tag="tp5")
        for j in range(hw):
            g = half * hw + j
            nc.tensor.transpose(tp[:, j, :], x2c[:, g * 128:(g + 1) * 128], identH[:])
        nc.vector.tensor_tensor(out=odb[:, gsl, :], in0=tp[:],
                                in1=x1[:, gsl, dsl], op=AL.add)
        nc.sync.dma_start(out=outv[:, gsl, :], in_=odb[:, gsl, :])


def _emit(nc, tc, ap, half):
    chunks = CHUNKS[half]
    own_rb = [rb for qc in chunks for rb in (2 * qc, 2 * qc + 1)]

    ctx = ExitStack()
    with ctx:
        persist = ctx.enter_context(tc.tile_pool(name="persist", bufs=1))

        eps_t = persist.tile([128, 1], F32)
        nc.vector.memset(eps_t, EPS / (SX * SX))
        identH = persist.tile([128, 128], F16)
        rotT = persist.tile([128, 128], F16)
        cosT = persist.tile([HD, S], F16)     # pre-scaled by 1/(SW*SX)
        sinT = persist.tile([HD, S], F16)
        maskT = persist.tile([128, 2, 2, CH], F32)

        gbig = ctx.enter_context(tc.tile_pool(name="gbig", bufs=1))
        x1 = gbig.tile([128, 4, D], F16)         # 2MB fp16 (own 512 rows)

        n2stats = gbig.tile([128, 4, DB, 6], F32)
        mw = ctx.enter_context(tc.tile_pool(name="mw", bufs=4))

        # ---- head loop pools (weight pool opened early for prefetch) ----
        hlA = ExitStack()
        wpool = hlA.enter_context(tc.tile_pool(name="wpool", bufs=3))
        small = hlA.enter_context(tc.tile_pool(name="small", bufs=2))
        kqps = hlA.enter_context(tc.tile_pool(name="kqps", bufs=4, space="PSUM"))
        avps = hlA.enter_context(tc.tile_pool(name="avps", bufs=1, space="PSUM"))
        opool = hlA.enter_context(tc.tile_pool(name="opool", bufs=1))
        oTh = opool.tile([128, H, 512], F8)      # 1MB (16*o, hi)
        oTl = opool.tile([128, H, 512], F8)      # 1MB (16*o, lo)
        wop = hlA.enter_context(tc.tile_pool(name="wop", bufs=3))
        xop = hlA.enter_context(tc.tile_pool(name="xop", bufs=4))
        wo_tiles = {}

        def wo_dma(db):
            wo_t = wop.tile([128, 2, H, 128], F8, tag="wo")
            nc.sync.dma_start(
                out=wo_t,
                in_=ap["wo"][db].rearrange("p (two hh m) -> p two hh m",
                                           two=2, m=128))
            wo_tiles[db] = wo_t

        gu_tiles = {}

        def gu_dma(fb):
            wg_t = mw.tile([128, 2, DB, 128], F8, tag="wg")
            wu_t = mw.tile([128, 2, DB, 128], F8, tag="wu")
            nc.sync.dma_start(out=wg_t, in_=ap["wg"][fb].rearrange(
                "p (two db m) -> p two db m", two=2, m=128))
            nc.sync.dma_start(out=wu_t, in_=ap["wu"][fb].rearrange(
                "p (two db m) -> p two db m", two=2, m=128))
            gu_tiles[fb] = (wg_t, wu_t)

        hlB = ExitStack()
        vps = hlB.enter_context(tc.tile_pool(name="vps", bufs=1, space="PSUM"))
        abig = hlB.enter_context(tc.tile_pool(name="abig", bufs=1))
        h1Th = abig.tile([128, DB, S], F8)       # 2MB (8*h1, hi)
        h1Tl = abig.tile([128, DB, S], F8)       # 2MB (8*h1, lo)

        wtiles = {}

        def wdma(h):
            wk_t = wpool.tile([128, 2, DB, 128], F8, tag="wk")
            wv_t = wpool.tile([128, 2, DB, 128], F8, tag="wv")
            wq_t = wpool.tile([128, 2, DB, 128], F8, tag="wq")
            for t, nm in ((wk_t, "wk"), (wv_t, "wv"), (wq_t, "wq")):
                nc.sync.dma_start(out=t, in_=ap[nm][h].rearrange(
                    "p (two db m) -> p two db m", two=2, m=128))
            wtiles[h] = (wk_t, wv_t, wq_t)

        def _kmm(g, wk_t):
            ssl = slice(g * 512, (g + 1) * 512)
            kp = kqps.tile([128, 512], F32, tag="kp")
            _dr(nc, kp[:], wk_t, h1Th, h1Tl, list(range(DB)), ssl,
                start=True, stop=False)
            tsin = small.tile([128, 512], F16, tag="tsin")
            nc.vector.tensor_tensor(out=tsin[:], in0=kp[:], in1=sinT[:, ssl], op=AL.mult)
            nc.vector.tensor_tensor(out=kp[:], in0=kp[:], in1=cosT[:, ssl], op=AL.mult)
            return kp, tsin

        def _vmm(vb, wv_t):
            # tokens stationary, weights moving -> natural [token, hd] layout
            vp = vps.tile([128, 4, 128], F32, tag="vp")
            for j in range(4):
                blk = vb * 4 + j
                bsl = slice(blk * 128, (blk + 1) * 128)
                for pi, (wi, xm) in enumerate([(0, h1Th), (1, h1Th), (0, h1Tl)]):
                    for t0 in range(0, DB, 2):
                        nc.tensor.matmul(
                            vp[:, j, :], xm[:, t0:t0 + 2, bsl],
                            wv_t[:, wi, t0:t0 + 2, :],
                            start=(pi == 0 and t0 == 0),
                            stop=(pi == 2 and t0 == DB - 2),
                            perf_mode=PM.DoubleRow, skip_group_check=True)
            return vp

        # ---- phase A: rmsnorm1 -> h1 (fp16, 8x scale) -> transpose ->
        #      split into h1Th/h1Tl fp8 ----
        pres = {0: {}, 1: {}, 2: {}}
        nc.sync.dma_start(out=identH, in_=ap["identH"])
        with tc.tile_pool(name="stA", bufs=2) as stA, \
             tc.tile_pool(name="stAx", bufs=1) as stAx, \
             tc.tile_pool(name="stAps", bufs=2, space="PSUM") as stAps:
            xown = {}
            for rb in [0, 1, 2, 3, "pre", 4, 5, "pre2", 6, 7]:
                if rb == "pre":
                    # head-0/1/2 partial work on tokens 0-511 fills the PE
                    # while the rest of phase A streams through DVE/Act/Pool
                    for hh in (0, 1):
                        pres[hh]["w"] = wtiles.pop(hh)
                        pres[hh]["kp0"], pres[hh]["tsin0"] = \
                            _kmm(0, pres[hh]["w"][0])
                    pres[0]["vp0"] = _vmm(0, pres[0]["w"][1])
                    continue
                if rb == "pre2":
                    pres[2]["w"] = wtiles.pop(2)
                    pres[2]["kp0"], pres[2]["tsin0"] = _kmm(0, pres[2]["w"][0])
                    continue
                if rb in own_rb:
                    x_t = xop.tile([128, D], F16, tag="xot")
                    xown[own_rb.index(rb)] = x_t
                else:
                    x_t = stA.tile([128, D], F16, tag="x_t")
                rsl_ = slice(rb * 128, (rb + 1) * 128)
                nc.sync.dma_start(out=x_t[:, 0:D // 2], in_=ap["xbh"][rsl_, 0:D // 2])
                nc.sync.dma_start(out=x_t[:, D // 2:D], in_=ap["xbh"][rsl_, D // 2:D])
                xsq = stAx.tile([128, D], F16, tag="xsq")
                acc = stA.tile([128, 2], F32, tag="acc")
                if rb == 1:
                    wdma(0)
                    nc.sync.dma_start(out=rotT, in_=ap["rotT"])
                    nc.sync.dma_start(out=cosT, in_=ap["cosT"])
                    nc.sync.dma_start(out=sinT, in_=ap["sinT"])
                elif rb == 2:
                    wdma(1)
                elif rb == 3:
                    nc.sync.dma_start(out=maskT,
                                      in_=ap["maskT"].rearrange("c k p q -> p c k q"))
                    wdma(2)
                for hx in range(2):
                    hsl_ = slice(hx * (D // 2), (hx + 1) * (D // 2))
                    nc.scalar.activation(out=xsq[:, hsl_], in_=x_t[:, hsl_],
                                         func=AF.Square,
                                         accum_out=acc[:, hx:hx + 1])
                accs = stA.tile([128, 1], F32, tag="accs")
                nc.vector.tensor_tensor(out=accs[:], in0=acc[:, 0:1],
                                        in1=acc[:, 1:2], op=AL.add)
                # sd = sqrt(mean(x^2)+eps)/SX ; rstd = SX/sd scale folded in
                sd = stA.tile([128, 1], F32, tag="sdA")
                nc.scalar.activation(out=sd[:], in_=accs[:], func=AF.Sqrt,
                                     scale=float(1.0 / (D * SX * SX)), bias=eps_t[:])
                rstd = stA.tile([128, 1], F32, tag="rstdA")
                nc.vector.reciprocal(out=rstd[:], in_=sd[:])
                h1 = stA.tile([128, D], F16, tag="h1")
                for hk in range(2):
                    hsl = slice(hk * 1024, (hk + 1) * 1024)
                    if hk == 0:
                        nc.scalar.mul(h1[:, hsl], x_t[:, hsl], rstd[:])
                    else:
                        nc.vector.tensor_scalar_mul(h1[:, hsl], x_t[:, hsl], rstd[:])
                    for pk in range(2 * hk, 2 * hk + 2):
                        tp = stAps.tile([128, 4, 128], F16, tag="tpA")
                        for j in range(4):
                            nc.tensor.transpose(
                                tp[:, j, :],
                                h1[:, (4 * pk + j) * 128:(4 * pk + j + 1) * 128],
                                identH[:])
                        rsl = slice(rb * 128, (rb + 1) * 128)
                        dsth = h1Th[:, 4 * pk:4 * pk + 4, rsl]
                        nc.scalar.copy(dsth, tp[:])
                        nc.vector.tensor_tensor(
                            out=h1Tl[:, 4 * pk:4 * pk + 4, rsl],
                            in0=tp[:], in1=dsth, op=AL.subtract)

        kpool = hlB.enter_context(tc.tile_pool(name="kpool", bufs=2))
        ppool = hlB.enter_context(tc.tile_pool(name="ppool", bufs=2))
        sps = hlB.enter_context(tc.tile_pool(name="sps", bufs=2, space="PSUM"))

        hstate = {}

        def _own_msl(qc):
            return slice(qc * CH, (qc + 1) * CH)

        def proj(h, pre=None, mid=None):
            if pre is None:
                wk_t, wv_t, wq_t = wtiles.pop(h)
            else:
                wk_t, wv_t, wq_t = pre["w"]

            kT_h = kpool.tile([128, S], F16, tag="kT")
            v_nat = kpool.tile([128, SB, 128], F16, tag="v_nat")
            qT_h = kpool.tile([128, 512], F16, tag="qT")

            def krot(g, kp, tsin):
                ssl = slice(g * 512, (g + 1) * 512)
                nc.tensor.matmul(kp[:], rotT[:], tsin[:],
                                 start=False, stop=True, skip_group_check=True)
                nc.scalar.copy(kT_h[:, ssl], kp[:])

            if pre is None:
                kp0, tsin0 = _kmm(0, wk_t)
                vp0 = _vmm(0, wv_t)
            else:
                kp0, tsin0 = pre["kp0"], pre["tsin0"]
                vp0 = pre.get("vp0")
                if vp0 is None:
                    vp0 = _vmm(0, wv_t)
            kp1, tsin1 = _kmm(1, wk_t)
            if mid is not None:
                mid()
            krot(0, kp0, tsin0)
            nc.scalar.mul(v_nat[:, 0:4, :], vp0[:], float(SXO / (SW * SX)))

            # q matmuls (own chunks), DoubleRow per chunk
            qp = kqps.tile([128, 512], F32, tag="kp")
            qpv = qp[:].rearrange("p (a c) -> p a c", c=CH)
            for ci, qc in enumerate(chunks):
                # one spanning accumulation group: only the very first matmul
                # carries start=True (bank-wide lazy zero covers chunk 1)
                _dr(nc, qpv[:, ci, :], wq_t, h1Th, h1Tl, list(range(DB)),
                    _own_msl(qc), start=(ci == 0), stop=False)
            krot(1, kp1, tsin1)
            tsin = small.tile([128, 512], F16, tag="tsin")
            tsv = tsin[:].rearrange("p (a c) -> p a c", c=CH)
            for ci, qc in enumerate(chunks):
                osl = _own_msl(qc)
                nc.vector.tensor_tensor(out=tsv[:, ci, :], in0=qpv[:, ci, :],
                                        in1=sinT[:, osl], op=AL.mult)
                nc.vector.tensor_tensor(out=qpv[:, ci, :], in0=qpv[:, ci, :],
                                        in1=cosT[:, osl], op=AL.mult)

            vp1 = _vmm(1, wv_t)
            nc.tensor.matmul(qp[:], rotT[:], tsin[:],
                             start=False, stop=True, skip_group_check=True)
            nc.scalar.copy(qT_h[:], qp[:])
            nc.vector.tensor_scalar_mul(v_nat[:, 4:8, :], vp1[:],
                                        float(SXO / (SW * SX)))
            if h + 3 < H:
                wdma(h + 3)
            if h >= 13:
                wo_dma(h - 13)
            hstate[h] = (kT_h, v_nat, qT_h)

        def attn_scores(h):
            kT_h, v_nat, qT_h = hstate.pop(h)
            pcs = []
            for ci, qc in enumerate(chunks):
                nkb = 2 * qc + 2
                osl = slice(ci * CH, (ci + 1) * CH)
                p_sb = ppool.tile([128, SB, CH], F16, tag="p_sb")
                for kb in range(nkb):
                    sp = sps.tile([128, CH], F32, tag="sp")
                    nc.tensor.matmul(sp[:], kT_h[:, kb * 128:(kb + 1) * 128],
                                     qT_h[:, osl], start=True, stop=True,
                                     skip_group_check=True)
                    j = kb - 2 * qc
                    if j >= 0:
                        nc.vector.scalar_tensor_tensor(
                            out=sp[:], in0=sp[:], scalar=ISQ,
                            in1=maskT[:, ci, j, :], op0=AL.mult, op1=AL.add)
                        nc.scalar.activation(out=p_sb[:, kb, :], in_=sp[:],
                                             func=AF.Exp)
                    else:
                        nc.scalar.activation(out=p_sb[:, kb, :], in_=sp[:],
                                             func=AF.Exp, scale=ISQ)
                den = small.tile([128, CH], F32, tag="den")
                nc.vector.tensor_tensor(out=den[:], in0=p_sb[:, 0, :],
                                        in1=p_sb[:, 1, :], op=AL.add)
                for kb in range(2, nkb):
                    nc.vector.tensor_tensor(out=den[:], in0=den[:],
                                            in1=p_sb[:, kb, :], op=AL.add)
                dall = small.tile([128, CH], F32, tag="dall")
                nc.gpsimd.partition_all_reduce(dall[:], den[:], 128,
                                               bass_isa.ReduceOp.add)
                dbc = small.tile([128, CH], F32, tag="dbc")
                nc.vector.reciprocal(out=dbc[:], in_=dall[:])
                pcs.append((nkb, osl, p_sb, dbc))
            return v_nat, pcs

        def attn_av(h, st):
            v_nat, pcs = st
            op_ = avps.tile([128, 2, CH], F32, tag="op")
            for ci in range(2):
                nkb, osl, p_sb, dbc = pcs[ci]
                for kb in range(nkb):
                    nc.tensor.matmul(op_[:, ci, :], v_nat[:, kb, :], p_sb[:, kb, :],
                                     start=(kb == 0), stop=(kb == nkb - 1),
                                     skip_group_check=True)
                t16 = small.tile([128, CH], F16, tag="t16")
                nc.vector.tensor_tensor(out=t16[:], in0=op_[:, ci, :],
                                        in1=dbc[:], op=AL.mult)
                nc.gpsimd.tensor_copy(oTh[:, h, osl], t16[:])
                nc.vector.tensor_tensor(out=oTl[:, h, osl], in0=t16[:],
                                        in1=oTh[:, h, osl], op=AL.subtract)

        ast = {}

        for h in range(H):
            def mid(hh=h):
                if hh > 0:
                    ast[hh - 1] = attn_scores(hh - 1)
            proj(h, pres.get(h) or None, mid=mid)
            if h > 0:
                attn_av(h - 1, ast.pop(h - 1))
        # start o-proj db0 on head-pairs 0..6 (heads 0-13 are finished) so
        # the PE keeps running while head 15's softmax/AV epilogue drains
        xp0 = kqps.tile([128, 512], F32, tag="kp")
        _dr(nc, xp0[:], wo_tiles[0], oTh, oTl, list(range(H - 2)), slice(None),
            start=True, stop=False)
        ast[H - 1] = attn_scores(H - 1)
        attn_av(H - 1, ast.pop(H - 1))
        hlB.close()

        # ---- o-proj + residual -> x1 (SBUF) + incremental norm2 stats ----
        with tc.tile_pool(name="st3", bufs=2) as st3:
            pending = None
            for db in range(DB):
                if db + 3 < DB:
                    wo_dma(db + 3)
                if db in (2, 6, 10, 13):
                    gu_dma({2: 0, 6: 1, 10: 2, 13: 3}[db])
                wo_t = wo_tiles.pop(db)
                if db == 0:
                    xp = xp0
                    _dr(nc, xp[:], wo_t, oTh, oTl, [H - 2], slice(None),
                        start=False, stop=True)
                else:
                    xp = kqps.tile([128, 512], F32, tag="kp")
                    _dr(nc, xp[:], wo_t, oTh, oTl, list(range(H)), slice(None))
                if pending is not None:
                    _finish_oproj(nc, avps, identH, x1, xown, n2stats, *pending)
                xoT = st3.tile([128, 512], F16, tag="xoT")
                pending = (db, xp, xoT)
            _finish_oproj(nc, avps, identH, x1, xown, n2stats, *pending)
        hlA.close()

        mlp = ctx.enter_context(tc.tile_pool(name="mlp", bufs=1))
        h2Th = mlp.tile([128, DB, 512], F8)      # 1MB (8*h2, hi)
        h2Tl = mlp.tile([128, DB, 512], F8)      # 1MB (8*h2, lo)
        hadh = mlp.tile([128, FFB, 512], F8)     # 4MB (8*had, hi)

        # ---- norm2 finalize -> h2Th/h2Tl (fp8, 8x scale) ----
        with tc.tile_pool(name="st4", bufs=4) as st4, \
             tc.tile_pool(name="nps", bufs=4, space="PSUM") as nps:
            diags = []
            for tg in range(4):
                mv = st4.tile([128, 2], F32, tag="n2mv")
                nc.vector.bn_aggr(out=mv[:], in_=n2stats[:, tg, :, :])
                msq = st4.tile([128, 1], F32, tag="n2msq")
                nc.vector.scalar_tensor_tensor(
                    out=msq[:], in0=mv[:, 0:1], scalar=mv[:, 0:1], in1=mv[:, 1:2],
                    op0=AL.mult, op1=AL.add)
                sd = st4.tile([128, 1], F32, tag="n2sd")
                nc.scalar.activation(out=sd[:], in_=msq[:], func=AF.Sqrt,
                                     scale=float(1.0 / (SX * SX)), bias=eps_t[:])
                rstd = st4.tile([128, 1], F32, tag="n2rstd")
                nc.vector.reciprocal(out=rstd[:], in_=sd[:])
                # diag(SX*rstd) fp16; x1_slice^T @ diag transposes + norm-scales
                diag = st4.tile([128, 128], F16, tag="n2diag")
                nc.vector.tensor_scalar_mul(diag[:], identH[:], rstd[:])
                diags.append(diag)
            # pk-major so h2T k-tiles complete in db order
            for pk in range(4):
                for tg in range(4):
                    tp = nps.tile([128, 4, 128], F32, tag="tpN")
                    for j in range(4):
                        nc.tensor.matmul(
                            tp[:, j, :],
                            x1[:, tg, (4 * pk + j) * 128:(4 * pk + j + 1) * 128],
                            diags[tg][:], start=True, stop=True,
                            skip_group_check=True)
                    tsl = slice(tg * 128, (tg + 1) * 128)
                    dsth = h2Th[:, 4 * pk:4 * pk + 4, tsl]
                    nc.scalar.copy(dsth, tp[:])
                    nc.vector.tensor_tensor(out=h2Tl[:, 4 * pk:4 * pk + 4, tsl],
                                            in0=tp[:], in1=dsth, op=AL.subtract)

        # ---- MLP gate/up -> hadh/hadl ----
        wdp_cm = tc.tile_pool(name="wd", bufs=2)
        wdp = wdp_cm.__enter__()
        wd_tiles = {}

        def wd_dma(db):
            wd_t = wdp.tile([128, 2, FFB, 128], F8, tag="wd")
            nc.sync.dma_start(out=wd_t, in_=ap["wd"][db].rearrange(
                "p (two fb m) -> p two fb m", two=2, m=128))
            wd_tiles[db] = wd_t

        with tc.tile_pool(name="mls", bufs=3) as mls, \
             tc.tile_pool(name="gps", bufs=2, space="PSUM") as gps, \
             tc.tile_pool(name="ups", bufs=2, space="PSUM") as ups:
            for fb in range(FFB):
                if fb + 4 < FFB:
                    gu_dma(fb + 4)
                elif fb == FFB - 4:
                    wd_dma(0)
                elif fb == FFB - 2:
                    wd_dma(1)
                wg_t, wu_t = gu_tiles.pop(fb)
                gp = gps.tile([128, 512], F32, tag="gp")
                up = ups.tile([128, 512], F32, tag="up")
                _dr(nc, gp[:], wg_t, h2Th, h2Tl, list(range(DB)), slice(None))
                _dr(nc, up[:], wu_t, h2Th, h2Tl, list(range(DB)), slice(None))
                sg = mls.tile([128, 512], F16, tag="sg")
                nc.scalar.activation(out=sg[:], in_=gp[:], func=AF.Silu,
                                     scale=float(1.0 / (SW * SX)))
                t16 = mls.tile([128, 512], F16, tag="t16m")
                nc.vector.scalar_tensor_tensor(
                    out=t16[:], in0=up[:], scalar=float(SXH / (SW * SX)),
                    in1=sg[:], op0=AL.mult, op1=AL.mult)
                nc.gpsimd.tensor_copy(hadh[:, fb, :], t16[:])
        # ---- MLP down + final residual -> out ----
        with tc.tile_pool(name="st5", bufs=2) as st5, \
             tc.tile_pool(name="outp", bufs=2) as outp, \
             tc.tile_pool(name="dps", bufs=2, space="PSUM") as dps, \
             tc.tile_pool(name="tps2", bufs=2, space="PSUM") as tps2:
            pending = None
            for db in range(DB):
                if db + 2 < DB:
                    wd_dma(db + 2)
                wd_t = wd_tiles.pop(db)
                dp = dps.tile([128, 512], F32, tag="dp")
                _dr(nc, dp[:], wd_t, hadh, None, list(range(FFB)), slice(None),
                    skip_xl=True)
                if pending is not None:
                    _finish_down(nc, tps2, identH, x1, outp, ap, *pending)
                x2c = st5.tile([128, 512], F16, tag="x2c")
                pending = (db, dp, x2c)
            _finish_down(nc, tps2, identH, x1, outp, ap, *pending, split=True)
        wdp_cm.__exit__(None, None, None)


def _build(half):
    nc = bacc.Bacc("TRN2", target_bir_lowering=False, debug=False, num_devices=8)
    ap = {}

    def din(name, shape, dt=F8):
        ap[name] = nc.dram_tensor(name, shape, dt, kind="ExternalInput").ap()

    din("xbh", [S, D], F16)
    din("cosT", [HD, S], F16)
    din("sinT", [HD, S], F16)
    din("maskT", [2, 2, 128, CH], F32)
    din("identH", [128, 128], F16)
    din("rotT", [128, 128], F16)
    din("wq", [H, 128, 2 * DB * 128]); din("wk", [H, 128, 2 * DB * 128])
    din("wv", [H, 128, 2 * DB * 128])
    din("wo", [DB, 128, 2 * H * 128])
    din("wg", [FFB, 128, 2 * DB * 128]); din("wu", [FFB, 128, 2 * DB * 128])
    din("wd", [DB, 128, 2 * FFB * 128])
    ap["out"] = nc.dram_tensor("out", [DB, 512, 128], F32, kind="ExternalOutput").ap()

    with tile.TileContext(nc) as tc:
        _emit(nc, tc, ap, half)
    nc.compile()
    return nc


def _q8(a):
    return np.asarray(a, np.float32).astype(ml_dtypes.float8_e4m3)


def _hilo(w, s):
    """[.., K-tiles, m] float32 -> stacked hi/lo fp8 at scale s (axis -3)."""
    wh = _q8(w * s)
    wl = _q8(w * s - wh.astype(np.float32))
    return np.stack([wh, wl], axis=-3)


def _prep(inputs):
    inp = {k: np.asarray(v) for k, v in inputs.items()}
    w1 = inp["norm_weight_1"].astype(np.float32)
    w2 = inp["norm_weight_2"].astype(np.float32)

    def fold(n):
        return (inp[f"w_{n}"].astype(np.float32)
                + inp[f"w_{n}_lora_a"].astype(np.float32)
                @ inp[f"w_{n}_lora_b"].astype(np.float32))

    ident = np.eye(128, dtype=np.float16)
    Rm = np.zeros((128, 128), np.float32)
    for i in range(64):
        Rm[i, i + 64] = -1.0
        Rm[i + 64, i] = 1.0

    def _colmajor8(w, nblk, s):
        # [K, N] -> [nblk, 128, 2*(K/128)*(N/nblk)] fp8 hi/lo
        K, N = w.shape
        r = (w.reshape(K // 128, 128, nblk, N // nblk)
             .transpose(2, 1, 0, 3))          # [nblk, 128, KT, n]
        hl = _hilo(r, s)                      # [nblk, 128, 2, KT, n]
        return np.ascontiguousarray(
            hl.reshape(nblk, 128, 2 * (K // 128) * (N // nblk)))

    wo_f = fold("o")
    wo_r = wo_f.reshape(H, 128, DB, 128).transpose(2, 1, 0, 3)  # [DB,128,H,128]
    wo_hl = np.ascontiguousarray(_hilo(wo_r, SW).reshape(DB, 128, 2 * H * 128))
    wd_f = fold("down")
    wd_r = wd_f.reshape(FFB, 128, DB, 128).transpose(2, 1, 0, 3)
    wd_hl = np.ascontiguousarray(_hilo(wd_r, SWD).reshape(DB, 128, 2 * FFB * 128))

    shared = dict(
        wq=_colmajor8(w1[:, None] * fold("q"), H, SW),
        wk=_colmajor8(w1[:, None] * fold("k"), H, SW),
        wv=_colmajor8(w1[:, None] * fold("v"), H, SW),
        wo=wo_hl,
        wg=_colmajor8(w2[:, None] * fold("gate"), FFB, SW),
        wu=_colmajor8(w2[:, None] * fold("up"), FFB, SW),
        wd=wd_hl,
        identH=ident, rotT=np.ascontiguousarray(Rm.T.astype(np.float16)))

    pos = inp["position_ids"].astype(np.int64)
    cs = float(1.0 / (SW * SX))
    cos_p = inp["cos"].astype(np.float32)[pos] * cs
    sin_p = inp["sin"].astype(np.float32)[pos] * cs
    mask = inp["attention_mask"].astype(np.float32)[0, 0]
    x = inp["x"].astype(np.float32)

    in_maps = []
    for c in range(8):
        b = c // 2
        half = c % 2
        mT = np.zeros((2, 2, 128, CH), np.float32)
        for ci, qc in enumerate(CHUNKS[half]):
            for j in range(2):
                kb = 2 * qc + j
                mT[ci, j] = mask[qc * CH:(qc + 1) * CH, kb * 128:(kb + 1) * 128].T
        m = dict(shared)
        m.update(xbh=np.ascontiguousarray(x[b].astype(np.float16)),
                 cosT=np.ascontiguousarray(cos_p[b].T.astype(np.float16)),
                 sinT=np.ascontiguousarray(sin_p[b].T.astype(np.float16)),
                 maskT=mT)
        in_maps.append(m)
    return in_maps


def kernel(**inputs):
    in_maps = _prep(inputs)
    if "nc" not in _CACHE:
        _CACHE["nc"] = (_build(0), _build(1))
    nc0, nc1 = _CACHE["nc"]

    res0 = run_bass_kernel_spmd(nc0, [in_maps[c] for c in (0, 2, 4, 6)],
                                core_ids=[0, 2, 4, 6])
    res1 = run_bass_kernel_spmd(nc1, [in_maps[c] for c in (1, 3, 5, 7)],
                                core_ids=[1, 3, 5, 7])

    out = np.zeros((B, S, D), np.float32)
    for res, halfi, cores in ((res0, 0, (0, 2, 4, 6)), (res1, 1, (1, 3, 5, 7))):
        for gi, c in enumerate(cores):
            b = c // 2
            r = res.results[gi]["out"]   # [DB, 512, 128]
            for ci, qc in enumerate(CHUNKS[halfi]):
                for db in range(DB):
                    out[b, qc * CH:(qc + 1) * CH, db * 128:(db + 1) * 128] = \
                        r[db, ci * CH:(ci + 1) * CH, :]
    return out
